# revision 19
# baseline (speedup 1.0000x reference)
"""Multi-head causal attention (B=4, S=2048, H=16, Dh=64, Dm=1024) on 8
Trainium2 NeuronCores.

Sharding: core c handles batch b = c//2 and heads [8*(c%2), 8*(c%2)+8).
Each core computes its 8 heads' full attention + O-projection partial sum;
the host adds the two half-head partials per batch plus O_b.

v2 layout (all matmul inputs bf16, PSUM f32):
  - Inputs land in SBUF as bf16 (half the HBM traffic of v1); loads are
    spread over the sync + scalar HWDGE queues and emitted first.
  - One flat emission stream keeps the PE dense: the attention j-loop for
    pair 0 starts as soon as its q/k projections are in, and the remaining
    projection / O-projection matmul groups are drip-fed as fillers
    between j iterations so the PE never idles (holds the high p-state).
  - Causal narrowing: for diagonal k-tiles only columns >= o are computed
    (logits matmul, exp, S@V); the 128-wide staircase strip is masked with
    one DVE multiply (bf16, perf-mode eligible).
  - Softmax: exp (ACT) is the only Scalar-engine work.  Denominators come
    from the ones-column of the v tiles (M=65 S@V); normalization is
    DVE reciprocal on the PSUM denominator row -> gpsimd partition
    broadcast -> two DVE multiplies; head B is repacked to partitions
    64:127 with one SBUF->SBUF DMA so the O-projection contracts K=128.
  - O-projection writes PSUM straight to DRAM y via sync-queue DMA.
"""

import os
import sys

sys.path.insert(0, "/opt/trn_rl_repo")

import numpy as np

B, S, DM, H, DH = 4, 2048, 1024, 16, 64
HPC = 8          # heads per core
NPAIR = HPC // 2
PB = 512         # q block width
NQP = S // PB    # 4 q blocks
MT = DM // 128   # 8 m-tiles
NKT = S // 128   # 16 k tiles

_cache = {}


def _split_multi_waits(nc, mybir):
    # This container's walrus rejects >1 sync wait per instruction
    # ("Too many sync wait commands").  Move extra waits onto same-engine
    # NoOps right before the instruction; per-engine program order makes
    # this equivalent.
    ctr = 0
    for fn in nc.m.functions:
        for blk in fn.blocks:
            insts = list(blk.instructions)
            new_insts = []
            changed = False
            for inst in insts:
                si = getattr(inst, "sync_info", None)
                waits = list(si.on_wait) if (si is not None and si.on_wait) else []
                if len(waits) > 1:
                    changed = True
                    for w in waits[:-1]:
                        ctr += 1
                        new_insts.append(
                            mybir.InstNoOp(
                                name=f"waitsplit-{ctr}",
                                engine=inst.engine,
                                ins=[],
                                outs=[],
                                sync_info=mybir.SyncInfo(on_wait=[w], on_update=[]),
                            )
                        )
                    si.on_wait = [waits[-1]]
                new_insts.append(inst)
            if changed:
                blk.instructions = new_insts


def _patch_tile_drain(tile_mod, bass_mod):
    # Same walrus limitation hits the Tile kernel-tail drain (one wait per
    # ticked proc).  Chain the waits through single-wait sync NoOps.
    from concourse.vector_clock import ScopedClock, VectorClock

    def _drain_and_barrier(self, tick_clock, wait_clock):
        gc = tick_clock.global_clock
        n = len(gc)
        ticks = [gc[i] for i in range(n)]
        for p in [i for i in range(n) if ticks[i] > 0]:
            nop = self.nc.sync.nop(nofuse=True, hint="drain_wait_split")
            vc = VectorClock([ticks[j] if j == p else 0 for j in range(n)])
            wait_clock.add_sem_waits(nop.ins, ScopedClock({None: vc}))
        self.nc.sync.drain()
        self.nc.all_engine_barrier()
        assert self.sems is not None
        popped = self.nc._tile_sem_poison_stack.pop()
        assert popped is self._sem_poison
        self.nc.clear_and_free_semaphores(list(self.sems.allocated().values()))
        self.nc.all_engine_barrier()

    tile_mod.TileContext._drain_and_barrier = _drain_and_barrier


def _build():
    if "nc" in _cache:
        return _cache["nc"]

    import concourse.bass as bass
    import concourse.mybir as mybir
    import concourse.tile as tile

    _patch_tile_drain(tile, bass)

    f32 = mybir.dt.float32
    f32r = mybir.dt.float32r
    bf16 = mybir.dt.bfloat16
    Exp = mybir.ActivationFunctionType.Exp

    nc = bass.Bass()
    xT = nc.dram_tensor("xT", [DM, S], bf16, kind="ExternalInput")
    Wq = nc.dram_tensor("Wq", [DM, 512], bf16, kind="ExternalInput")
    Wk = nc.dram_tensor("Wk", [DM, 512], bf16, kind="ExternalInput")
    Wv = nc.dram_tensor("Wv", [DM, 512], bf16, kind="ExternalInput")
    Wo = nc.dram_tensor("Wo", [512, DM], bf16, kind="ExternalInput")
    qkb = nc.dram_tensor("qkb", [128, 8], f32, kind="ExternalInput")
    vbb = nc.dram_tensor("vbb", [128, 512], f32, kind="ExternalInput")
    stair2 = nc.dram_tensor("stair2", [128, 256], bf16, kind="ExternalInput")
    onz = nc.dram_tensor("onz", [128, 8], bf16, kind="ExternalInput")
    one64 = nc.dram_tensor("one64", [1, 64], f32r, kind="ExternalInput")
    y = nc.dram_tensor("y", [S, DM], f32, kind="ExternalOutput")
    dd1s = [nc.dram_tensor(f"dd1_{k}", [1024], f32, kind="Internal")
            for k in range(2)]
    dd2s = [nc.dram_tensor(f"dd2_{k}", [1024], f32, kind="Internal")
            for k in range(2)]

    with tile.TileContext(nc) as tc:
        with nc.allow_low_precision(reason="bf16 tiles feeding the PE"), \
             tc.tile_pool(name="mp", bufs=1) as mp, \
             tc.tile_pool(name="sp", bufs=1) as sp, \
             tc.tile_pool(name="pp", bufs=1, space="PSUM") as pp:

            # ---- constants (gpsimd queue; tiny) ----
            qkb_sb = mp.tile([128, 8], f32, tag="qkb")
            nc.gpsimd.dma_start(qkb_sb[:], qkb[:])
            vbb_sb = mp.tile([128, 512], f32, tag="vbb")
            nc.gpsimd.dma_start(vbb_sb[:], vbb[:])
            stair_sb = mp.tile([128, 256], bf16, tag="stair")
            nc.gpsimd.dma_start(stair_sb[:], stair2[:])
            onz_sb = mp.tile([128, 8], bf16, tag="onz")
            nc.gpsimd.dma_start(onz_sb[:], onz[:])
            one64_sb = mp.tile([1, 64], f32r, tag="one64")
            nc.gpsimd.dma_start(one64_sb[:], one64[:])

            # ---- input loads: weights on scalar HWDGE, x on sync HWDGE ----
            wv = []
            for m in range(MT):
                w = mp.tile([128, 512], bf16, tag=f"wv{m}")
                nc.scalar.dma_start(w[:], Wv[m * 128:(m + 1) * 128, :])
                wv.append(w)
            wq = []
            wk = []
            for m in range(MT):
                w = mp.tile([128, 512], bf16, tag=f"wq{m}")
                nc.scalar.dma_start(w[:], Wq[m * 128:(m + 1) * 128, :])
                wq.append(w)
                w = mp.tile([128, 512], bf16, tag=f"wk{m}")
                nc.scalar.dma_start(w[:], Wk[m * 128:(m + 1) * 128, :])
                wk.append(w)
            xt = []
            for m in range(MT):
                t = mp.tile([128, S], bf16, tag=f"xt{m}")
                nc.sync.dma_start(t[:], xT[m * 128:(m + 1) * 128, :])
                xt.append(t)
            wo = []
            for pri in range(NPAIR):
                t = mp.tile([128, DM], bf16, tag=f"wo{pri}")
                nc.gpsimd.dma_start(t[:], Wo[pri * 128:(pri + 1) * 128, :])
                wo.append(t)

            # ---- persistent result tiles ----
            # v: [p, h*65+d] per 128-row k-tile; col 65h+64 = ones so the
            # merged S@V matmul (M=65) also produces the softmax denominator
            v_sb = [mp.tile([128, 520], bf16, tag=f"v{p}", name=f"v{p}")
                    for p in range(NKT)]
            qkT = {(t, pri): mp.tile([128, S], bf16, tag=f"{t}T{pri}",
                                     name=f"{t}T{pri}")
                   for t in ("q", "k") for pri in range(NPAIR)}
            at_sb = {(pri, i): mp.tile([128, 512], bf16, tag=f"at{pri}_{i}",
                                       name=f"at{pri}_{i}")
                     for pri in range(NPAIR) for i in range(NQP)}

            # ---- filler units (each ~0.9-1.9us of PE work) ----
            def unit_vproj(p):
                def emit():
                    ps = pp.tile([128, 512], f32, tag="proj", bufs=2)
                    for m in range(MT):
                        nc.tensor.matmul(
                            ps[:], xt[m][:, p * 128:(p + 1) * 128], wv[m][:],
                            start=(m == 0), stop=(m == MT - 1))
                    vt = v_sb[p]
                    nc.vector.tensor_add(
                        vt.rearrange("p (h c) -> p h c", c=65)[:, :, 0:64],
                        ps.rearrange("p (h c) -> p h c", c=64),
                        vbb_sb.rearrange("p (h c) -> p h c", c=64))
                    nc.gpsimd.tensor_copy(
                        vt.rearrange("p (h c) -> p h c", c=65)[:, :, 64:65],
                        onz_sb.rearrange("p (h c) -> p h c", c=1))
                return emit

            def unit_qkproj(ti, pri, pb):
                def emit():
                    W = wq if ti == 0 else wk
                    out = qkT[("q" if ti == 0 else "k", pri)]
                    ps = pp.tile([128, 512], f32, tag="proj", bufs=2)
                    for m in range(MT):
                        nc.tensor.matmul(
                            ps[:], W[m][:, pri * 128:(pri + 1) * 128],
                            xt[m][:, pb * 512:(pb + 1) * 512],
                            start=(m == 0), stop=(m == MT - 1))
                    nc.vector.tensor_scalar_add(
                        out[:, pb * 512:(pb + 1) * 512], ps[:],
                        qkb_sb[:, 4 * ti + pri:4 * ti + pri + 1])
                return emit

            def unit_oproj(i, pt, dm):
                def emit():
                    P = 4 * i + pt
                    ps = pp.tile([128, 512], f32, tag="proj", bufs=2)
                    for pri in range(NPAIR):
                        nc.tensor.matmul(
                            ps[:],
                            at_sb[(pri, i)][:, pt * 128:(pt + 1) * 128],
                            wo[pri][:, dm * 512:(dm + 1) * 512],
                            start=(pri == 0), stop=(pri == NPAIR - 1))
                    yt = sp.tile([128, 512], f32, tag="yt", bufs=6, name="yt")
                    nc.scalar.copy(yt[:], ps[:])
                    nc.sync.dma_start(
                        y[P * 128:(P + 1) * 128, dm * 512:(dm + 1) * 512],
                        yt[:])
                return emit

            from collections import deque
            fillers = deque()   # (key, emit_fn, req); keys track emission
            chainq = deque()    # deferred normalization-chain ops
            emitted = set()
            chain_emitted = [0]

            def pop_chain():
                chainq.popleft()()
                chain_emitted[0] += 1

            def pop_filler():
                key, fn, req = fillers[0]
                # a filler may read tiles written by deferred chain ops;
                # force-emit the chain up to its snapshot first
                while chain_emitted[0] < req and chainq:
                    pop_chain()
                fillers.popleft()
                fn()
                emitted.add(key)

            def drain_until(keys):
                # engines run their queues in emission order, so a unit
                # producing data for block (pri, i) must be EMITTED before
                # the block's first consumer instruction
                while fillers and not keys <= emitted:
                    pop_filler()

            # preamble compute: v tiles 0..3 + pair-0 q/k block 0
            for p in range(4):
                unit_vproj(p)()
                emitted.add(("v", p))
            unit_qkproj(0, 0, 0)()
            unit_qkproj(1, 0, 0)()
            emitted.update({("q", 0, 0), ("k", 0, 0)})
            # everything else pair 0 needs, in dependency order for i=1..3
            fillers.extend((("v", p), unit_vproj(p), 0) for p in (4, 5))
            fillers.append((("q", 0, 1), unit_qkproj(0, 0, 1), 0))
            fillers.append((("k", 0, 1), unit_qkproj(1, 0, 1), 0))
            fillers.extend((("v", p), unit_vproj(p), 0)
                           for p in (6, 7, 8, 9))
            fillers.append((("q", 0, 2), unit_qkproj(0, 0, 2), 0))
            fillers.append((("k", 0, 2), unit_qkproj(1, 0, 2), 0))
            fillers.extend((("v", p), unit_vproj(p), 0)
                           for p in (10, 11, 12, 13))
            fillers.append((("q", 0, 3), unit_qkproj(0, 0, 3), 0))
            fillers.append((("k", 0, 3), unit_qkproj(1, 0, 3), 0))
            fillers.extend((("v", p), unit_vproj(p), 0) for p in (14, 15))

            # ---- attention: pair-outer, q-block inner ----
            for pri in range(NPAIR):
                if pri < NPAIR - 1:
                    for pb in range(NQP):
                        fillers.append((("q", pri + 1, pb),
                                        unit_qkproj(0, pri + 1, pb), 0))
                        fillers.append((("k", pri + 1, pb),
                                        unit_qkproj(1, pri + 1, pb), 0))
                qT = qkT[("q", pri)]
                kT = qkT[("k", pri)]
                for i in range(NQP):
                    kmax = 4 * (i + 1)
                    drain_until({("q", pri, i)}
                                | {("k", pri, pb) for pb in range(i + 1)}
                                | {("v", p) for p in range(kmax)})
                    if i == 0 and fillers:
                        pop_filler()
                    ad = pp.tile([65, 1024], f32, tag="ad", bufs=1)

                    def emit_logits(j):
                        o = (j - 4 * i) * 128 if j >= 4 * i else 0
                        ev = pp.tile([128, 1024], f32, tag="ev", bufs=2,
                                     name="ev")
                        for h in range(2):
                            nc.tensor.matmul(
                                ev[:, h * 512 + o:(h + 1) * 512],
                                kT[64 * h:64 * h + 64, j * 128:(j + 1) * 128],
                                qT[64 * h:64 * h + 64,
                                   i * 512 + o:(i + 1) * 512],
                                start=True, stop=True)
                        return ev, o

                    # logits run two j's ahead so the ACT exp stream never
                    # starves across interleaved filler matmuls
                    evq = {0: emit_logits(0)}
                    if kmax > 1:
                        evq[1] = emit_logits(1)
                    for j in range(kmax):
                        ev, o = evq.pop(j)
                        sc = sp.tile([128, 1024], bf16, tag="sc", bufs=8)
                        if o:
                            nc.scalar.activation(
                                sc.rearrange("p (h c) -> p h c",
                                             c=512)[:, :, o:],
                                ev.rearrange("p (h c) -> p h c",
                                             c=512)[:, :, o:],
                                Exp, scale=0.125)
                        else:
                            nc.scalar.activation(sc[:], ev[:], Exp,
                                                 scale=0.125)
                        if j >= 4 * i:
                            # staircase mask on the 128-wide diagonal strip
                            # (both heads in one DVE multiply)
                            strip = sc.rearrange(
                                "p (h c) -> p h c", c=512)[:, :, o:o + 128]
                            nc.vector.tensor_mul(
                                strip, strip,
                                stair_sb.rearrange("p (h c) -> p h c", c=128))
                        st = (j == 0)
                        sp_ = (j == kmax - 1)
                        vt = v_sb[j]
                        for h in range(2):
                            lh = 2 * pri + h
                            nc.tensor.matmul(
                                ad[0:65, h * 512 + o:(h + 1) * 512],
                                vt[:, lh * 65:lh * 65 + 65],
                                sc[:, h * 512 + o:(h + 1) * 512],
                                start=st, stop=sp_, skip_group_check=True)
                        if j + 2 < kmax:
                            evq[j + 2] = emit_logits(j + 2)
                        if chainq:
                            pop_chain()
                        if fillers and (j % 2 == 1):
                            pop_filler()
                    # normalization: evacuate ad via ACT (releases the PSUM
                    # bank fast), then reciprocal + broadcast of the
                    # denominator row, then two multiplies + head-B repack
                    final = (pri == NPAIR - 1 and i == NQP - 1)
                    adc = sp.tile([65, 1024], f32, tag="adc", bufs=4,
                                  name="adc")
                    nc.scalar.copy(adc[:], ad[:, :])
                    at = at_sb[(pri, i)]
                    tmp = sp.tile([64, 512], bf16, tag="tmp", bufs=4)
                    if final:
                        # tail chain: DVE/PE are idle, so the slow one-lane
                        # reciprocal + PE outer-product broadcast beat the
                        # DMA round trip's latency
                        rec = sp.tile([1, 1024], f32r, tag="rec", bufs=1)
                        nc.vector.reciprocal(rec[:], adc[64:65, :])
                        bc = pp.tile([128, 1024], f32, tag="ev", bufs=2,
                                     name="bc")
                        for hh in range(2):
                            nc.tensor.matmul(
                                bc[0:64, hh * 512:(hh + 1) * 512],
                                one64_sb[:], rec[:, hh * 512:(hh + 1) * 512],
                                start=True, stop=True)
                        nc.vector.tensor_mul(at[0:64, :], adc[0:64, 0:512],
                                             bc[0:64, 0:512])
                        nc.vector.tensor_mul(tmp[:], adc[0:64, 512:1024],
                                             bc[0:64, 512:1024])
                        nc.vector.tensor_mul(at[0:64, :], adc[0:64, 0:512],
                                             bc[0:64, 0:512])
                        nc.vector.tensor_mul(tmp[:], adc[0:64, 512:1024],
                                             bc[0:64, 512:1024])
                        nc.sync.dma_start(at[64:128, :], tmp[:])
                    else:
                        # spread the 1024 denominators over 128 DVE lanes
                        # with a DRAM round trip.  Each chain op is deferred
                        # into the NEXT block's j-loop (one per j) so its
                        # input is already in flight when it reaches the
                        # head of its engine queue - no head-of-line stalls
                        # on the engines serving the attention j-chain.
                        dd1 = dd1s[(4 * pri + i) % 2]
                        dd2 = dd2s[(4 * pri + i) % 2]
                        dn = sp.tile([128, 8], f32, tag="dn", bufs=4,
                                     name="dn")
                        rT = sp.tile([128, 8], f32r, tag="rT", bufs=4,
                                     name="rT")
                        bcs = sp.tile([64, 1024], f32, tag="bcs", bufs=4,
                                      name="bcs")

                        def chain_ops(adc=adc, at=at, tmp=tmp, dd1=dd1,
                                      dd2=dd2, dn=dn, rT=rT, bcs=bcs):
                            yield lambda: nc.gpsimd.dma_start(
                                dd1.rearrange("(o f) -> o f", o=1),
                                adc[64:65, :])
                            yield lambda: nc.sync.dma_start(
                                dn[:], dd1.rearrange("(c p) -> p c", p=128))
                            yield lambda: nc.vector.reciprocal(rT[:], dn[:])
                            yield lambda: nc.gpsimd.dma_start(
                                dd2.rearrange("(c p) -> p c", p=128),
                                rT.bitcast(f32)[:])
                            yield lambda: nc.sync.dma_start(
                                bcs[:],
                                dd2.rearrange("(o f) -> o f",
                                              o=1).partition_broadcast(64))
                            yield lambda: nc.vector.tensor_mul(
                                at[0:64, :], adc[0:64, 0:512], bcs[:, 0:512])
                            yield lambda: nc.gpsimd.tensor_mul(
                                tmp[:], adc[0:64, 512:1024], bcs[:, 512:1024])
                            yield lambda: nc.sync.dma_start(at[64:128, :],
                                                            tmp[:])

                        chainq.extend(chain_ops())
                    if pri == NPAIR - 1:
                        req = chain_emitted[0] + len(chainq)
                        for dm in range(2):
                            for pt in range(4):
                                fillers.append((("o", i, pt, dm),
                                                unit_oproj(i, pt, dm), req))
            while chainq:
                pop_chain()
            while fillers:
                pop_filler()

    _split_multi_waits(nc, mybir)
    _cache["nc"] = nc
    return nc


def _host_inputs(x, Q_w, Q_b, K_w, K_b, V_w, V_b, O_w):
    import ml_dtypes
    bf = ml_dtypes.bfloat16
    stair = (np.arange(128)[:, None] <= np.arange(128)[None, :]).astype(bf)
    stair2 = np.concatenate([stair, stair], axis=1)
    in_maps = []
    for c in range(8):
        b, hs = c // 2, HPC * (c % 2)
        he = hs + HPC
        qb = Q_b[hs:he].reshape(512).astype(np.float32)
        kb = K_b[hs:he].reshape(512).astype(np.float32)
        qkb = np.zeros((128, 8), np.float32)
        for pri in range(NPAIR):
            qkb[:, pri] = qb[pri * 128:(pri + 1) * 128]
            qkb[:, 4 + pri] = kb[pri * 128:(pri + 1) * 128]
        in_maps.append({
            "xT": np.ascontiguousarray(x[b].T).astype(bf),
            "Wq": np.ascontiguousarray(
                Q_w[hs:he].transpose(1, 0, 2).reshape(DM, 512)).astype(bf),
            "Wk": np.ascontiguousarray(
                K_w[hs:he].transpose(1, 0, 2).reshape(DM, 512)).astype(bf),
            "Wv": np.ascontiguousarray(
                V_w[hs:he].transpose(1, 0, 2).reshape(DM, 512)).astype(bf),
            "Wo": np.ascontiguousarray(O_w[hs:he].reshape(512, DM)).astype(bf),
            "qkb": qkb,
            "vbb": np.tile(V_b[hs:he].reshape(1, 512), (128, 1)).astype(
                np.float32),
            "stair2": stair2,
            "onz": np.ones((128, 8), bf),
            "one64": np.ones((1, 64), np.float32),
        })
    return in_maps


def kernel(x, Q_w, Q_b, K_w, K_b, V_w, V_b, O_w, O_b, _trace=False):
    x = np.asarray(x, np.float32)
    args = [np.asarray(a, np.float32)
            for a in (Q_w, Q_b, K_w, K_b, V_w, V_b, O_w)]
    O_b = np.asarray(O_b, np.float32)

    nc = _build()
    from concourse.bass_utils import run_bass_kernel_spmd

    in_maps = _host_inputs(x, *args)
    res = run_bass_kernel_spmd(nc, in_maps, core_ids=list(range(8)),
                               trace=_trace)
    _cache["last_result"] = res
    out = np.empty((B, S, DM), np.float32)
    for b in range(B):
        out[b] = res.results[2 * b]["y"] + res.results[2 * b + 1]["y"] + O_b
    return out


if __name__ == "__main__":
    # quick self-run with random inputs
    rng = np.random.default_rng(0)
    x = rng.standard_normal((B, S, DM), dtype=np.float32)
    shp = dict(Q_w=(H, DM, DH), Q_b=(H, DH), K_w=(H, DM, DH), K_b=(H, DH),
               V_w=(H, DM, DH), V_b=(H, DH), O_w=(H, DH, DM), O_b=(DM,))
    ins = {k: rng.standard_normal(v, dtype=np.float32) * 0.05
           for k, v in shp.items()}
    out = kernel(x, **ins)
    print("ran", out.shape, out.dtype)


# revision 24
# speedup vs baseline: 1.3930x; 1.3930x over previous
"""Multi-head causal attention (B=4, S=2048, H=16, Dh=64, Dm=1024) on 8
Trainium2 NeuronCores.

Sharding: core c handles batch b = c//2 and heads [8*(c%2), 8*(c%2)+8).
Each core computes its 8 heads' full attention + O-projection partial sum;
the host adds the two half-head partials per batch plus O_b.

v2 layout (all matmul inputs bf16, PSUM f32):
  - Inputs land in SBUF as bf16 (half the HBM traffic of v1); loads are
    spread over the sync + scalar HWDGE queues and emitted first.
  - One flat emission stream keeps the PE dense: the attention j-loop for
    pair 0 starts as soon as its q/k projections are in, and the remaining
    projection / O-projection matmul groups are drip-fed as fillers
    between j iterations so the PE never idles (holds the high p-state).
  - Causal narrowing: for diagonal k-tiles only columns >= o are computed
    (logits matmul, exp, S@V); the 128-wide staircase strip is masked with
    one DVE multiply (bf16, perf-mode eligible).
  - Softmax: exp (ACT) is the only Scalar-engine work.  Denominators come
    from the ones-column of the v tiles (M=65 S@V); normalization is
    DVE reciprocal on the PSUM denominator row -> gpsimd partition
    broadcast -> two DVE multiplies; head B is repacked to partitions
    64:127 with one SBUF->SBUF DMA so the O-projection contracts K=128.
  - O-projection writes PSUM straight to DRAM y via sync-queue DMA.
"""

import os
import sys

sys.path.insert(0, "/opt/trn_rl_repo")

import numpy as np

B, S, DM, H, DH = 4, 2048, 1024, 16, 64
HPC = 8          # heads per core
NPAIR = HPC // 2
PB = 512         # q block width
NQP = S // PB    # 4 q blocks
MT = DM // 128   # 8 m-tiles
NKT = S // 128   # 16 k tiles

_cache = {}


def _split_multi_waits(nc, mybir):
    # This container's walrus rejects >1 sync wait per instruction
    # ("Too many sync wait commands").  Move extra waits onto same-engine
    # NoOps right before the instruction; per-engine program order makes
    # this equivalent.
    ctr = 0
    for fn in nc.m.functions:
        for blk in fn.blocks:
            insts = list(blk.instructions)
            new_insts = []
            changed = False
            for inst in insts:
                si = getattr(inst, "sync_info", None)
                waits = list(si.on_wait) if (si is not None and si.on_wait) else []
                if len(waits) > 1:
                    changed = True
                    for w in waits[:-1]:
                        ctr += 1
                        new_insts.append(
                            mybir.InstNoOp(
                                name=f"waitsplit-{ctr}",
                                engine=inst.engine,
                                ins=[],
                                outs=[],
                                sync_info=mybir.SyncInfo(on_wait=[w], on_update=[]),
                            )
                        )
                    si.on_wait = [waits[-1]]
                new_insts.append(inst)
            if changed:
                blk.instructions = new_insts


def _patch_tile_drain(tile_mod, bass_mod):
    # Same walrus limitation hits the Tile kernel-tail drain (one wait per
    # ticked proc).  Chain the waits through single-wait sync NoOps.
    from concourse.vector_clock import ScopedClock, VectorClock

    def _drain_and_barrier(self, tick_clock, wait_clock):
        gc = tick_clock.global_clock
        n = len(gc)
        ticks = [gc[i] for i in range(n)]
        for p in [i for i in range(n) if ticks[i] > 0]:
            nop = self.nc.sync.nop(nofuse=True, hint="drain_wait_split")
            vc = VectorClock([ticks[j] if j == p else 0 for j in range(n)])
            wait_clock.add_sem_waits(nop.ins, ScopedClock({None: vc}))
        self.nc.sync.drain()
        self.nc.all_engine_barrier()
        assert self.sems is not None
        popped = self.nc._tile_sem_poison_stack.pop()
        assert popped is self._sem_poison
        self.nc.clear_and_free_semaphores(list(self.sems.allocated().values()))
        self.nc.all_engine_barrier()

    tile_mod.TileContext._drain_and_barrier = _drain_and_barrier


def _build():
    if "nc" in _cache:
        return _cache["nc"]

    import concourse.bass as bass
    import concourse.mybir as mybir
    import concourse.tile as tile

    _patch_tile_drain(tile, bass)

    f32 = mybir.dt.float32
    f32r = mybir.dt.float32r
    bf16 = mybir.dt.bfloat16
    Exp = mybir.ActivationFunctionType.Exp
    Recip = mybir.ActivationFunctionType.Reciprocal

    def act_reciprocal(out, in_):
        # softmax denominators are well-conditioned positives; the ACT
        # reciprocal's reduced accuracy costs ~0.1% here and avoids the
        # cross-partition DMA round trip entirely
        eng = nc.scalar
        return eng.add_instruction(
            mybir.InstActivation(
                name=nc.get_next_instruction_name(),
                func=Recip,
                ins=[eng.lower_ap(in_),
                     mybir.ImmediateValue(dtype=f32, value=0.0),
                     mybir.ImmediateValue(dtype=f32, value=1.0),
                     mybir.ImmediateValue(dtype=f32, value=0.0)],
                outs=[eng.lower_ap(out)],
            ))

    nc = bass.Bass()
    xT = nc.dram_tensor("xT", [DM, S], bf16, kind="ExternalInput")
    Wq = nc.dram_tensor("Wq", [DM, 512], bf16, kind="ExternalInput")
    Wk = nc.dram_tensor("Wk", [DM, 512], bf16, kind="ExternalInput")
    Wv = nc.dram_tensor("Wv", [DM, 512], bf16, kind="ExternalInput")
    Wo = nc.dram_tensor("Wo", [512, DM], bf16, kind="ExternalInput")
    qkb = nc.dram_tensor("qkb", [128, 8], f32, kind="ExternalInput")
    vbb = nc.dram_tensor("vbb", [128, 512], f32, kind="ExternalInput")
    stair2 = nc.dram_tensor("stair2", [128, 256], bf16, kind="ExternalInput")
    onz = nc.dram_tensor("onz", [128, 8], bf16, kind="ExternalInput")
    one64 = nc.dram_tensor("one64", [1, 64], f32r, kind="ExternalInput")
    y = nc.dram_tensor("y", [S, DM], f32, kind="ExternalOutput")
    dd1s = [nc.dram_tensor(f"dd1_{k}", [1024], f32, kind="Internal")
            for k in range(2)]
    dd2s = [nc.dram_tensor(f"dd2_{k}", [1024], f32, kind="Internal")
            for k in range(2)]

    with tile.TileContext(nc) as tc:
        with nc.allow_low_precision(reason="bf16 tiles feeding the PE"), \
             tc.tile_pool(name="mp", bufs=1) as mp, \
             tc.tile_pool(name="sp", bufs=1) as sp, \
             tc.tile_pool(name="pp", bufs=1, space="PSUM") as pp:

            # ---- constants (gpsimd queue; tiny) ----
            qkb_sb = mp.tile([128, 8], f32, tag="qkb")
            nc.gpsimd.dma_start(qkb_sb[:], qkb[:])
            vbb_sb = mp.tile([128, 512], f32, tag="vbb")
            nc.gpsimd.dma_start(vbb_sb[:], vbb[:])
            stair_sb = mp.tile([128, 256], bf16, tag="stair")
            nc.gpsimd.dma_start(stair_sb[:], stair2[:])
            onz_sb = mp.tile([128, 8], bf16, tag="onz")
            nc.gpsimd.dma_start(onz_sb[:], onz[:])
            one64_sb = mp.tile([1, 64], f32r, tag="one64")
            nc.gpsimd.dma_start(one64_sb[:], one64[:])

            # ---- input loads: weights on scalar HWDGE, x on sync HWDGE ----
            wv = []
            for m in range(MT):
                w = mp.tile([128, 512], bf16, tag=f"wv{m}")
                nc.scalar.dma_start(w[:], Wv[m * 128:(m + 1) * 128, :])
                wv.append(w)
            wq = []
            wk = []
            for m in range(MT):
                w = mp.tile([128, 512], bf16, tag=f"wq{m}")
                nc.scalar.dma_start(w[:], Wq[m * 128:(m + 1) * 128, :])
                wq.append(w)
                w = mp.tile([128, 512], bf16, tag=f"wk{m}")
                nc.scalar.dma_start(w[:], Wk[m * 128:(m + 1) * 128, :])
                wk.append(w)
            xt = []
            for m in range(MT):
                t = mp.tile([128, S], bf16, tag=f"xt{m}")
                nc.sync.dma_start(t[:], xT[m * 128:(m + 1) * 128, :])
                xt.append(t)
            wo = []
            for pri in range(NPAIR):
                t = mp.tile([128, DM], bf16, tag=f"wo{pri}")
                nc.gpsimd.dma_start(t[:], Wo[pri * 128:(pri + 1) * 128, :])
                wo.append(t)

            # ---- persistent result tiles ----
            # v: [p, h*65+d] per 128-row k-tile; col 65h+64 = ones so the
            # merged S@V matmul (M=65) also produces the softmax denominator
            v_sb = [mp.tile([128, 520], bf16, tag=f"v{p}", name=f"v{p}")
                    for p in range(NKT)]
            qkT = {(t, pri): mp.tile([128, S], bf16, tag=f"{t}T{pri}",
                                     name=f"{t}T{pri}")
                   for t in ("q", "k") for pri in range(NPAIR)}
            at_sb = {(pri, i): mp.tile([128, 512], bf16, tag=f"at{pri}_{i}",
                                       name=f"at{pri}_{i}")
                     for pri in range(NPAIR) for i in range(NQP)}

            # ---- filler units (each ~0.9-1.9us of PE work) ----
            def unit_vproj(p):
                def emit():
                    ps = pp.tile([128, 512], f32, tag="proj", bufs=2)
                    for m in range(MT):
                        nc.tensor.matmul(
                            ps[:], xt[m][:, p * 128:(p + 1) * 128], wv[m][:],
                            start=(m == 0), stop=(m == MT - 1))
                    vt = v_sb[p]
                    nc.vector.tensor_add(
                        vt.rearrange("p (h c) -> p h c", c=65)[:, :, 0:64],
                        ps.rearrange("p (h c) -> p h c", c=64),
                        vbb_sb.rearrange("p (h c) -> p h c", c=64))
                    nc.gpsimd.tensor_copy(
                        vt.rearrange("p (h c) -> p h c", c=65)[:, :, 64:65],
                        onz_sb.rearrange("p (h c) -> p h c", c=1))
                return emit

            def unit_qkproj(ti, pri, pb):
                def emit():
                    W = wq if ti == 0 else wk
                    out = qkT[("q" if ti == 0 else "k", pri)]
                    ps = pp.tile([128, 512], f32, tag="proj", bufs=2)
                    for m in range(MT):
                        nc.tensor.matmul(
                            ps[:], W[m][:, pri * 128:(pri + 1) * 128],
                            xt[m][:, pb * 512:(pb + 1) * 512],
                            start=(m == 0), stop=(m == MT - 1))
                    nc.vector.tensor_scalar_add(
                        out[:, pb * 512:(pb + 1) * 512], ps[:],
                        qkb_sb[:, 4 * ti + pri:4 * ti + pri + 1])
                return emit

            def unit_oproj(i, pt, dm):
                def emit():
                    P = 4 * i + pt
                    ps = pp.tile([128, 512], f32, tag="proj", bufs=2)
                    for pri in range(NPAIR):
                        nc.tensor.matmul(
                            ps[:],
                            at_sb[(pri, i)][:, pt * 128:(pt + 1) * 128],
                            wo[pri][:, dm * 512:(dm + 1) * 512],
                            start=(pri == 0), stop=(pri == NPAIR - 1))
                    yt = sp.tile([128, 512], f32, tag="yt", bufs=6, name="yt")
                    nc.scalar.copy(yt[:], ps[:])
                    nc.sync.dma_start(
                        y[P * 128:(P + 1) * 128, dm * 512:(dm + 1) * 512],
                        yt[:])
                return emit

            from collections import deque
            fillers = deque()   # (key, emit_fn, req); keys track emission
            chainq = deque()    # deferred normalization-chain ops
            emitted = set()
            chain_emitted = [0]

            def pop_chain():
                chainq.popleft()()
                chain_emitted[0] += 1

            def pop_filler():
                key, fn, req = fillers[0]
                # a filler may read tiles written by deferred chain ops;
                # force-emit the chain up to its snapshot first
                while chain_emitted[0] < req and chainq:
                    pop_chain()
                fillers.popleft()
                fn()
                emitted.add(key)

            def drain_until(keys):
                # engines run their queues in emission order, so a unit
                # producing data for block (pri, i) must be EMITTED before
                # the block's first consumer instruction
                while fillers and not keys <= emitted:
                    pop_filler()

            # preamble compute: v tiles 0..3 + pair-0 q/k block 0
            for p in range(4):
                unit_vproj(p)()
                emitted.add(("v", p))
            unit_qkproj(0, 0, 0)()
            unit_qkproj(1, 0, 0)()
            emitted.update({("q", 0, 0), ("k", 0, 0)})
            # everything else pair 0 needs, in dependency order for i=1..3
            fillers.extend((("v", p), unit_vproj(p), 0) for p in (4, 5))
            fillers.append((("q", 0, 1), unit_qkproj(0, 0, 1), 0))
            fillers.append((("k", 0, 1), unit_qkproj(1, 0, 1), 0))
            fillers.extend((("v", p), unit_vproj(p), 0)
                           for p in (6, 7, 8, 9))
            fillers.append((("q", 0, 2), unit_qkproj(0, 0, 2), 0))
            fillers.append((("k", 0, 2), unit_qkproj(1, 0, 2), 0))
            fillers.extend((("v", p), unit_vproj(p), 0)
                           for p in (10, 11, 12, 13))
            fillers.append((("q", 0, 3), unit_qkproj(0, 0, 3), 0))
            fillers.append((("k", 0, 3), unit_qkproj(1, 0, 3), 0))
            fillers.extend((("v", p), unit_vproj(p), 0) for p in (14, 15))

            # ---- attention: pair-outer, q-block inner ----
            for pri in range(NPAIR):
                if pri < NPAIR - 1:
                    for pb in range(NQP):
                        fillers.append((("q", pri + 1, pb),
                                        unit_qkproj(0, pri + 1, pb), 0))
                        fillers.append((("k", pri + 1, pb),
                                        unit_qkproj(1, pri + 1, pb), 0))
                qT = qkT[("q", pri)]
                kT = qkT[("k", pri)]
                for i in range(NQP):
                    kmax = 4 * (i + 1)
                    drain_until({("q", pri, i)}
                                | {("k", pri, pb) for pb in range(i + 1)}
                                | {("v", p) for p in range(kmax)})
                    if i == 0 and fillers:
                        pop_filler()
                    ad = pp.tile([65, 1024], f32, tag="ad", bufs=1)

                    def emit_logits(j):
                        o = (j - 4 * i) * 128 if j >= 4 * i else 0
                        ev = pp.tile([128, 1024], f32, tag="ev", bufs=2,
                                     name="ev")
                        for h in range(2):
                            nc.tensor.matmul(
                                ev[:, h * 512 + o:(h + 1) * 512],
                                kT[64 * h:64 * h + 64, j * 128:(j + 1) * 128],
                                qT[64 * h:64 * h + 64,
                                   i * 512 + o:(i + 1) * 512],
                                start=True, stop=True)
                        return ev, o

                    # logits run two j's ahead so the ACT exp stream never
                    # starves across interleaved filler matmuls
                    evq = {0: emit_logits(0)}
                    if kmax > 1:
                        evq[1] = emit_logits(1)
                    for j in range(kmax):
                        ev, o = evq.pop(j)
                        sc = sp.tile([128, 1024], bf16, tag="sc", bufs=8)
                        if o:
                            nc.scalar.activation(
                                sc.rearrange("p (h c) -> p h c",
                                             c=512)[:, :, o:],
                                ev.rearrange("p (h c) -> p h c",
                                             c=512)[:, :, o:],
                                Exp, scale=0.125)
                        else:
                            nc.scalar.activation(sc[:], ev[:], Exp,
                                                 scale=0.125)
                        if j >= 4 * i:
                            # staircase mask on the 128-wide diagonal strip
                            # (both heads in one DVE multiply)
                            strip = sc.rearrange(
                                "p (h c) -> p h c", c=512)[:, :, o:o + 128]
                            nc.vector.tensor_mul(
                                strip, strip,
                                stair_sb.rearrange("p (h c) -> p h c", c=128))
                        st = (j == 0)
                        sp_ = (j == kmax - 1)
                        vt = v_sb[j]
                        for h in range(2):
                            lh = 2 * pri + h
                            nc.tensor.matmul(
                                ad[0:65, h * 512 + o:(h + 1) * 512],
                                vt[:, lh * 65:lh * 65 + 65],
                                sc[:, h * 512 + o:(h + 1) * 512],
                                start=st, stop=sp_, skip_group_check=True)
                        if j + 2 < kmax:
                            evq[j + 2] = emit_logits(j + 2)
                        if chainq:
                            pop_chain()
                        if fillers and (j % 2 == 1):
                            pop_filler()
                    # normalization: evacuate ad via ACT (releases the PSUM
                    # bank fast), then reciprocal + broadcast of the
                    # denominator row, then two multiplies + head-B repack
                    final = (pri == NPAIR - 1 and i == NQP - 1)
                    adc = sp.tile([65, 1024], f32, tag="adc", bufs=4,
                                  name="adc")
                    nc.scalar.copy(adc[:], ad[:, :])
                    at = at_sb[(pri, i)]
                    tmp = sp.tile([64, 512], bf16, tag="tmp", bufs=4)
                    # ACT hardware reciprocal of the denominator row (in the
                    # exp stream, right after the adc evacuation), then a PE
                    # outer-product broadcast and two DVE multiplies.  The
                    # PE/DVE ops are deferred into the next block's j-loop
                    # so they never head-of-line-block their engine queues.
                    rec = sp.tile([1, 1024], f32r, tag="rec", bufs=4,
                                  name="rec")
                    act_reciprocal(rec[:], adc[64:65, :])

                    def chain_ops(adc=adc, at=at, tmp=tmp, rec=rec):
                        bch = [pp.tile([128, 512], f32, tag="proj", bufs=2,
                                       name=f"bc{hh}") for hh in range(2)]

                        def bc_mm(hh):
                            return lambda: nc.tensor.matmul(
                                bch[hh][0:64, :], one64_sb[:],
                                rec[:, hh * 512:(hh + 1) * 512],
                                start=True, stop=True)
                        yield bc_mm(0)
                        yield bc_mm(1)
                        yield lambda: nc.vector.tensor_mul(
                            at[0:64, :], adc[0:64, 0:512], bch[0][0:64, :])
                        yield lambda: nc.vector.tensor_mul(
                            tmp[:], adc[0:64, 512:1024], bch[1][0:64, :])
                        yield lambda: nc.sync.dma_start(at[64:128, :],
                                                        tmp[:])

                    chainq.extend(chain_ops())
                    if pri == NPAIR - 1:
                        req = chain_emitted[0] + len(chainq)
                        for dm in range(2):
                            for pt in range(4):
                                fillers.append((("o", i, pt, dm),
                                                unit_oproj(i, pt, dm), req))
            while chainq:
                pop_chain()
            while fillers:
                pop_filler()

    _split_multi_waits(nc, mybir)
    _cache["nc"] = nc
    return nc


def _host_inputs(x, Q_w, Q_b, K_w, K_b, V_w, V_b, O_w):
    import ml_dtypes
    bf = ml_dtypes.bfloat16
    stair = (np.arange(128)[:, None] <= np.arange(128)[None, :]).astype(bf)
    stair2 = np.concatenate([stair, stair], axis=1)
    in_maps = []
    for c in range(8):
        b, hs = c // 2, HPC * (c % 2)
        he = hs + HPC
        qb = Q_b[hs:he].reshape(512).astype(np.float32)
        kb = K_b[hs:he].reshape(512).astype(np.float32)
        qkb = np.zeros((128, 8), np.float32)
        for pri in range(NPAIR):
            qkb[:, pri] = qb[pri * 128:(pri + 1) * 128]
            qkb[:, 4 + pri] = kb[pri * 128:(pri + 1) * 128]
        in_maps.append({
            "xT": np.ascontiguousarray(x[b].T).astype(bf),
            "Wq": np.ascontiguousarray(
                Q_w[hs:he].transpose(1, 0, 2).reshape(DM, 512)).astype(bf),
            "Wk": np.ascontiguousarray(
                K_w[hs:he].transpose(1, 0, 2).reshape(DM, 512)).astype(bf),
            "Wv": np.ascontiguousarray(
                V_w[hs:he].transpose(1, 0, 2).reshape(DM, 512)).astype(bf),
            "Wo": np.ascontiguousarray(O_w[hs:he].reshape(512, DM)).astype(bf),
            "qkb": qkb,
            "vbb": np.tile(V_b[hs:he].reshape(1, 512), (128, 1)).astype(
                np.float32),
            "stair2": stair2,
            "onz": np.ones((128, 8), bf),
            "one64": np.ones((1, 64), np.float32),
        })
    return in_maps


def kernel(x, Q_w, Q_b, K_w, K_b, V_w, V_b, O_w, O_b, _trace=False):
    x = np.asarray(x, np.float32)
    args = [np.asarray(a, np.float32)
            for a in (Q_w, Q_b, K_w, K_b, V_w, V_b, O_w)]
    O_b = np.asarray(O_b, np.float32)

    nc = _build()
    from concourse.bass_utils import run_bass_kernel_spmd

    in_maps = _host_inputs(x, *args)
    res = run_bass_kernel_spmd(nc, in_maps, core_ids=list(range(8)),
                               trace=_trace)
    _cache["last_result"] = res
    out = np.empty((B, S, DM), np.float32)
    for b in range(B):
        out[b] = res.results[2 * b]["y"] + res.results[2 * b + 1]["y"] + O_b
    return out


if __name__ == "__main__":
    # quick self-run with random inputs
    rng = np.random.default_rng(0)
    x = rng.standard_normal((B, S, DM), dtype=np.float32)
    shp = dict(Q_w=(H, DM, DH), Q_b=(H, DH), K_w=(H, DM, DH), K_b=(H, DH),
               V_w=(H, DM, DH), V_b=(H, DH), O_w=(H, DH, DM), O_b=(DM,))
    ins = {k: rng.standard_normal(v, dtype=np.float32) * 0.05
           for k, v in shp.items()}
    out = kernel(x, **ins)
    print("ran", out.shape, out.dtype)


# revision 26
# speedup vs baseline: 1.4703x; 1.0555x over previous
"""Multi-head causal attention (B=4, S=2048, H=16, Dh=64, Dm=1024) on 8
Trainium2 NeuronCores.

Sharding: core c handles batch b = c//2 and heads [8*(c%2), 8*(c%2)+8).
Each core computes its 8 heads' full attention + O-projection partial sum;
the host adds the two half-head partials per batch plus O_b.

v2 layout (all matmul inputs bf16, PSUM f32):
  - Inputs land in SBUF as bf16 (half the HBM traffic of v1); loads are
    spread over the sync + scalar HWDGE queues and emitted first.
  - One flat emission stream keeps the PE dense: the attention j-loop for
    pair 0 starts as soon as its q/k projections are in, and the remaining
    projection / O-projection matmul groups are drip-fed as fillers
    between j iterations so the PE never idles (holds the high p-state).
  - Causal narrowing: for diagonal k-tiles only columns >= o are computed
    (logits matmul, exp, S@V); the 128-wide staircase strip is masked with
    one DVE multiply (bf16, perf-mode eligible).
  - Softmax: exp (ACT) is the only Scalar-engine work.  Denominators come
    from the ones-column of the v tiles (M=65 S@V); normalization is
    DVE reciprocal on the PSUM denominator row -> gpsimd partition
    broadcast -> two DVE multiplies; head B is repacked to partitions
    64:127 with one SBUF->SBUF DMA so the O-projection contracts K=128.
  - O-projection writes PSUM straight to DRAM y via sync-queue DMA.
"""

import os
import sys

sys.path.insert(0, "/opt/trn_rl_repo")

import numpy as np

B, S, DM, H, DH = 4, 2048, 1024, 16, 64
HPC = 8          # heads per core
NPAIR = HPC // 2
PB = 512         # q block width
NQP = S // PB    # 4 q blocks
MT = DM // 128   # 8 m-tiles
NKT = S // 128   # 16 k tiles

_cache = {}


def _split_multi_waits(nc, mybir):
    # This container's walrus rejects >1 sync wait per instruction
    # ("Too many sync wait commands").  Move extra waits onto same-engine
    # NoOps right before the instruction; per-engine program order makes
    # this equivalent.
    ctr = 0
    for fn in nc.m.functions:
        for blk in fn.blocks:
            insts = list(blk.instructions)
            new_insts = []
            changed = False
            for inst in insts:
                si = getattr(inst, "sync_info", None)
                waits = list(si.on_wait) if (si is not None and si.on_wait) else []
                if len(waits) > 1:
                    changed = True
                    for w in waits[:-1]:
                        ctr += 1
                        new_insts.append(
                            mybir.InstNoOp(
                                name=f"waitsplit-{ctr}",
                                engine=inst.engine,
                                ins=[],
                                outs=[],
                                sync_info=mybir.SyncInfo(on_wait=[w], on_update=[]),
                            )
                        )
                    si.on_wait = [waits[-1]]
                new_insts.append(inst)
            if changed:
                blk.instructions = new_insts


def _patch_tile_drain(tile_mod, bass_mod):
    # Same walrus limitation hits the Tile kernel-tail drain (one wait per
    # ticked proc).  Chain the waits through single-wait sync NoOps.
    from concourse.vector_clock import ScopedClock, VectorClock

    def _drain_and_barrier(self, tick_clock, wait_clock):
        gc = tick_clock.global_clock
        n = len(gc)
        ticks = [gc[i] for i in range(n)]
        for p in [i for i in range(n) if ticks[i] > 0]:
            nop = self.nc.sync.nop(nofuse=True, hint="drain_wait_split")
            vc = VectorClock([ticks[j] if j == p else 0 for j in range(n)])
            wait_clock.add_sem_waits(nop.ins, ScopedClock({None: vc}))
        self.nc.sync.drain()
        self.nc.all_engine_barrier()
        assert self.sems is not None
        popped = self.nc._tile_sem_poison_stack.pop()
        assert popped is self._sem_poison
        self.nc.clear_and_free_semaphores(list(self.sems.allocated().values()))
        self.nc.all_engine_barrier()

    tile_mod.TileContext._drain_and_barrier = _drain_and_barrier


def _build():
    if "nc" in _cache:
        return _cache["nc"]

    import concourse.bass as bass
    import concourse.mybir as mybir
    import concourse.tile as tile

    _patch_tile_drain(tile, bass)

    f32 = mybir.dt.float32
    f32r = mybir.dt.float32r
    bf16 = mybir.dt.bfloat16
    Exp = mybir.ActivationFunctionType.Exp
    Ln = mybir.ActivationFunctionType.Ln

    nc = bass.Bass()
    xT = nc.dram_tensor("xT", [DM, S], bf16, kind="ExternalInput")
    Wq = nc.dram_tensor("Wq", [DM, 512], bf16, kind="ExternalInput")
    Wk = nc.dram_tensor("Wk", [DM, 512], bf16, kind="ExternalInput")
    Wv = nc.dram_tensor("Wv", [DM, 512], bf16, kind="ExternalInput")
    Wo = nc.dram_tensor("Wo", [512, DM], bf16, kind="ExternalInput")
    qkb = nc.dram_tensor("qkb", [128, 8], f32, kind="ExternalInput")
    vbb = nc.dram_tensor("vbb", [128, 512], f32, kind="ExternalInput")
    stair2 = nc.dram_tensor("stair2", [128, 256], bf16, kind="ExternalInput")
    onz = nc.dram_tensor("onz", [128, 8], bf16, kind="ExternalInput")
    one64 = nc.dram_tensor("one64", [1, 64], f32r, kind="ExternalInput")
    y = nc.dram_tensor("y", [S, DM], f32, kind="ExternalOutput")
    dd1s = [nc.dram_tensor(f"dd1_{k}", [1024], f32, kind="Internal")
            for k in range(2)]
    dd2s = [nc.dram_tensor(f"dd2_{k}", [1024], f32, kind="Internal")
            for k in range(2)]

    with tile.TileContext(nc) as tc:
        with nc.allow_low_precision(reason="bf16 tiles feeding the PE"), \
             tc.tile_pool(name="mp", bufs=1) as mp, \
             tc.tile_pool(name="sp", bufs=1) as sp, \
             tc.tile_pool(name="pp", bufs=1, space="PSUM") as pp:

            # ---- constants (gpsimd queue; tiny) ----
            qkb_sb = mp.tile([128, 8], f32, tag="qkb")
            nc.gpsimd.dma_start(qkb_sb[:], qkb[:])
            vbb_sb = mp.tile([128, 512], f32, tag="vbb")
            nc.gpsimd.dma_start(vbb_sb[:], vbb[:])
            stair_sb = mp.tile([128, 256], bf16, tag="stair")
            nc.gpsimd.dma_start(stair_sb[:], stair2[:])
            onz_sb = mp.tile([128, 8], bf16, tag="onz")
            nc.gpsimd.dma_start(onz_sb[:], onz[:])
            one64_sb = mp.tile([1, 64], f32r, tag="one64")
            nc.gpsimd.dma_start(one64_sb[:], one64[:])

            # ---- input loads: weights on scalar HWDGE, x on sync HWDGE ----
            wv = []
            for m in range(MT):
                w = mp.tile([128, 512], bf16, tag=f"wv{m}")
                nc.scalar.dma_start(w[:], Wv[m * 128:(m + 1) * 128, :])
                wv.append(w)
            wq = []
            wk = []
            for m in range(MT):
                w = mp.tile([128, 512], bf16, tag=f"wq{m}")
                nc.gpsimd.dma_start(w[:], Wq[m * 128:(m + 1) * 128, :])
                wq.append(w)
                w = mp.tile([128, 512], bf16, tag=f"wk{m}")
                nc.gpsimd.dma_start(w[:], Wk[m * 128:(m + 1) * 128, :])
                wk.append(w)
            xt = []
            for m in range(MT):
                t = mp.tile([128, S], bf16, tag=f"xt{m}")
                eng = nc.sync if m % 2 == 0 else nc.scalar
                eng.dma_start(t[:], xT[m * 128:(m + 1) * 128, :])
                xt.append(t)
            wo = []
            for pri in range(NPAIR):
                t = mp.tile([128, DM], bf16, tag=f"wo{pri}")
                nc.gpsimd.dma_start(t[:], Wo[pri * 128:(pri + 1) * 128, :])
                wo.append(t)

            # ---- persistent result tiles ----
            # v: [p, h*65+d] per 128-row k-tile; col 65h+64 = ones so the
            # merged S@V matmul (M=65) also produces the softmax denominator
            v_sb = [mp.tile([128, 520], bf16, tag=f"v{p}", name=f"v{p}")
                    for p in range(NKT)]
            qkT = {(t, pri): mp.tile([128, S], bf16, tag=f"{t}T{pri}",
                                     name=f"{t}T{pri}")
                   for t in ("q", "k") for pri in range(NPAIR)}
            at_sb = {(pri, i): mp.tile([128, 512], bf16, tag=f"at{pri}_{i}",
                                       name=f"at{pri}_{i}")
                     for pri in range(NPAIR) for i in range(NQP)}

            # ---- filler units (each ~0.9-1.9us of PE work) ----
            def unit_vproj(p):
                def emit():
                    ps = pp.tile([128, 512], f32, tag="proj", bufs=2)
                    for m in range(MT):
                        nc.tensor.matmul(
                            ps[:], xt[m][:, p * 128:(p + 1) * 128], wv[m][:],
                            start=(m == 0), stop=(m == MT - 1))
                    vt = v_sb[p]
                    nc.vector.tensor_add(
                        vt.rearrange("p (h c) -> p h c", c=65)[:, :, 0:64],
                        ps.rearrange("p (h c) -> p h c", c=64),
                        vbb_sb.rearrange("p (h c) -> p h c", c=64))
                    nc.gpsimd.tensor_copy(
                        vt.rearrange("p (h c) -> p h c", c=65)[:, :, 64:65],
                        onz_sb.rearrange("p (h c) -> p h c", c=1))
                return emit

            def unit_qkproj(ti, pri, pb):
                def emit():
                    W = wq if ti == 0 else wk
                    out = qkT[("q" if ti == 0 else "k", pri)]
                    ps = pp.tile([128, 512], f32, tag="proj", bufs=2)
                    for m in range(MT):
                        nc.tensor.matmul(
                            ps[:], W[m][:, pri * 128:(pri + 1) * 128],
                            xt[m][:, pb * 512:(pb + 1) * 512],
                            start=(m == 0), stop=(m == MT - 1))
                    nc.vector.tensor_scalar_add(
                        out[:, pb * 512:(pb + 1) * 512], ps[:],
                        qkb_sb[:, 4 * ti + pri:4 * ti + pri + 1])
                return emit

            def unit_oproj(i, pt, dm):
                def emit():
                    P = 4 * i + pt
                    ps = pp.tile([128, 512], f32, tag="proj", bufs=2)
                    for pri in range(NPAIR):
                        nc.tensor.matmul(
                            ps[:],
                            at_sb[(pri, i)][:, pt * 128:(pt + 1) * 128],
                            wo[pri][:, dm * 512:(dm + 1) * 512],
                            start=(pri == 0), stop=(pri == NPAIR - 1))
                    yt = sp.tile([128, 512], f32, tag="yt", bufs=6, name="yt")
                    nc.vector.tensor_copy(yt[:], ps[:])
                    nc.sync.dma_start(
                        y[P * 128:(P + 1) * 128, dm * 512:(dm + 1) * 512],
                        yt[:])
                return emit

            from collections import deque
            fillers = deque()   # (key, emit_fn, req); keys track emission
            chainq = deque()    # deferred normalization-chain ops
            emitted = set()
            chain_emitted = [0]

            def pop_chain():
                chainq.popleft()()
                chain_emitted[0] += 1

            def pop_filler():
                key, fn, req = fillers[0]
                # a filler may read tiles written by deferred chain ops;
                # force-emit the chain up to its snapshot first
                while chain_emitted[0] < req and chainq:
                    pop_chain()
                fillers.popleft()
                fn()
                emitted.add(key)

            def drain_until(keys):
                # engines run their queues in emission order, so a unit
                # producing data for block (pri, i) must be EMITTED before
                # the block's first consumer instruction
                while fillers and not keys <= emitted:
                    pop_filler()

            # preamble compute: v tiles 0..3 + pair-0 q/k block 0
            for p in range(4):
                unit_vproj(p)()
                emitted.add(("v", p))
            unit_qkproj(0, 0, 0)()
            unit_qkproj(1, 0, 0)()
            emitted.update({("q", 0, 0), ("k", 0, 0)})
            # everything else pair 0 needs, in dependency order for i=1..3
            fillers.extend((("v", p), unit_vproj(p), 0) for p in (4, 5))
            fillers.append((("q", 0, 1), unit_qkproj(0, 0, 1), 0))
            fillers.append((("k", 0, 1), unit_qkproj(1, 0, 1), 0))
            fillers.extend((("v", p), unit_vproj(p), 0)
                           for p in (6, 7, 8, 9))
            fillers.append((("q", 0, 2), unit_qkproj(0, 0, 2), 0))
            fillers.append((("k", 0, 2), unit_qkproj(1, 0, 2), 0))
            fillers.extend((("v", p), unit_vproj(p), 0)
                           for p in (10, 11, 12, 13))
            fillers.append((("q", 0, 3), unit_qkproj(0, 0, 3), 0))
            fillers.append((("k", 0, 3), unit_qkproj(1, 0, 3), 0))
            fillers.extend((("v", p), unit_vproj(p), 0) for p in (14, 15))

            # ---- attention: pair-outer, q-block inner ----
            for pri in range(NPAIR):
                if pri < NPAIR - 1:
                    for pb in range(NQP):
                        fillers.append((("q", pri + 1, pb),
                                        unit_qkproj(0, pri + 1, pb), 0))
                        fillers.append((("k", pri + 1, pb),
                                        unit_qkproj(1, pri + 1, pb), 0))
                qT = qkT[("q", pri)]
                kT = qkT[("k", pri)]
                for i in range(NQP):
                    kmax = 4 * (i + 1)
                    drain_until({("q", pri, i)}
                                | {("k", pri, pb) for pb in range(i + 1)}
                                | {("v", p) for p in range(kmax)})
                    if i == 0 and fillers:
                        pop_filler()
                    ad = pp.tile([65, 1024], f32, tag="ad", bufs=1)

                    def emit_logits(j):
                        o = (j - 4 * i) * 128 if j >= 4 * i else 0
                        ev = pp.tile([128, 1024], f32, tag="ev", bufs=2,
                                     name="ev")
                        for h in range(2):
                            nc.tensor.matmul(
                                ev[:, h * 512 + o:(h + 1) * 512],
                                kT[64 * h:64 * h + 64, j * 128:(j + 1) * 128],
                                qT[64 * h:64 * h + 64,
                                   i * 512 + o:(i + 1) * 512],
                                start=True, stop=True)
                        return ev, o

                    # logits run two j's ahead so the ACT exp stream never
                    # starves across interleaved filler matmuls
                    evq = {0: emit_logits(0)}
                    if kmax > 1:
                        evq[1] = emit_logits(1)
                    for j in range(kmax):
                        ev, o = evq.pop(j)
                        sc = sp.tile([128, 1024], bf16, tag="sc", bufs=8)
                        if o:
                            nc.scalar.activation(
                                sc.rearrange("p (h c) -> p h c",
                                             c=512)[:, :, o:],
                                ev.rearrange("p (h c) -> p h c",
                                             c=512)[:, :, o:],
                                Exp, scale=0.125)
                        else:
                            nc.scalar.activation(sc[:], ev[:], Exp,
                                                 scale=0.125)
                        if j >= 4 * i:
                            # staircase mask on the 128-wide diagonal strip
                            # (both heads in one DVE multiply)
                            strip = sc.rearrange(
                                "p (h c) -> p h c", c=512)[:, :, o:o + 128]
                            nc.vector.tensor_mul(
                                strip, strip,
                                stair_sb.rearrange("p (h c) -> p h c", c=128))
                        st = (j == 0)
                        sp_ = (j == kmax - 1)
                        vt = v_sb[j]
                        for h in range(2):
                            lh = 2 * pri + h
                            nc.tensor.matmul(
                                ad[0:65, h * 512 + o:(h + 1) * 512],
                                vt[:, lh * 65:lh * 65 + 65],
                                sc[:, h * 512 + o:(h + 1) * 512],
                                start=st, stop=sp_, skip_group_check=True)
                        if j + 2 < kmax:
                            evq[j + 2] = emit_logits(j + 2)
                        if chainq:
                            pop_chain()
                        if fillers and (j % 2 == 1):
                            pop_filler()
                    # normalization: evacuate ad via ACT (releases the PSUM
                    # bank fast), then reciprocal + broadcast of the
                    # denominator row, then two multiplies + head-B repack
                    final = (pri == NPAIR - 1 and i == NQP - 1)
                    adc = sp.tile([65, 1024], f32, tag="adc", bufs=4,
                                  name="adc")
                    nc.scalar.copy(adc[:], ad[:, :])
                    at = at_sb[(pri, i)]
                    tmp = sp.tile([64, 512], bf16, tag="tmp", bufs=4)
                    # ACT hardware reciprocal of the denominator row (in the
                    # exp stream, right after the adc evacuation), then a PE
                    # outer-product broadcast and two DVE multiplies.  The
                    # PE/DVE ops are deferred into the next block's j-loop
                    # so they never head-of-line-block their engine queues.
                    # 1/den as exp(-ln(den)): Ln and Exp live in the same
                    # ACT table set as the softmax exp, so no table reloads
                    lnt = sp.tile([1, 1024], f32, tag="lnt", bufs=2,
                                  name="lnt")
                    nc.scalar.activation(lnt[:], adc[64:65, :], Ln)
                    rec = sp.tile([1, 1024], f32r, tag="rec", bufs=2,
                                  name="rec")
                    nc.scalar.activation(rec[:], lnt[:], Exp, scale=-1.0)

                    def chain_ops(adc=adc, at=at, tmp=tmp, rec=rec):
                        bch = [pp.tile([128, 512], f32, tag="proj", bufs=2,
                                       name=f"bc{hh}") for hh in range(2)]

                        def bc_mm(hh):
                            return lambda: nc.tensor.matmul(
                                bch[hh][0:64, :], one64_sb[:],
                                rec[:, hh * 512:(hh + 1) * 512],
                                start=True, stop=True)
                        yield bc_mm(0)
                        yield bc_mm(1)
                        yield lambda: nc.vector.tensor_mul(
                            at[0:64, :], adc[0:64, 0:512], bch[0][0:64, :])
                        yield lambda: nc.vector.tensor_mul(
                            tmp[:], adc[0:64, 512:1024], bch[1][0:64, :])
                        yield lambda: nc.sync.dma_start(at[64:128, :],
                                                        tmp[:])

                    chainq.extend(chain_ops())
                    if pri == NPAIR - 1:
                        req = chain_emitted[0] + len(chainq)
                        for dm in range(2):
                            for pt in range(4):
                                fillers.append((("o", i, pt, dm),
                                                unit_oproj(i, pt, dm), req))
            while chainq:
                pop_chain()
            while fillers:
                pop_filler()

    _split_multi_waits(nc, mybir)
    _cache["nc"] = nc
    return nc


def _host_inputs(x, Q_w, Q_b, K_w, K_b, V_w, V_b, O_w):
    import ml_dtypes
    bf = ml_dtypes.bfloat16
    stair = (np.arange(128)[:, None] <= np.arange(128)[None, :]).astype(bf)
    stair2 = np.concatenate([stair, stair], axis=1)
    in_maps = []
    for c in range(8):
        b, hs = c // 2, HPC * (c % 2)
        he = hs + HPC
        qb = Q_b[hs:he].reshape(512).astype(np.float32)
        kb = K_b[hs:he].reshape(512).astype(np.float32)
        qkb = np.zeros((128, 8), np.float32)
        for pri in range(NPAIR):
            qkb[:, pri] = qb[pri * 128:(pri + 1) * 128]
            qkb[:, 4 + pri] = kb[pri * 128:(pri + 1) * 128]
        in_maps.append({
            "xT": np.ascontiguousarray(x[b].T).astype(bf),
            "Wq": np.ascontiguousarray(
                Q_w[hs:he].transpose(1, 0, 2).reshape(DM, 512)).astype(bf),
            "Wk": np.ascontiguousarray(
                K_w[hs:he].transpose(1, 0, 2).reshape(DM, 512)).astype(bf),
            "Wv": np.ascontiguousarray(
                V_w[hs:he].transpose(1, 0, 2).reshape(DM, 512)).astype(bf),
            "Wo": np.ascontiguousarray(O_w[hs:he].reshape(512, DM)).astype(bf),
            "qkb": qkb,
            "vbb": np.tile(V_b[hs:he].reshape(1, 512), (128, 1)).astype(
                np.float32),
            "stair2": stair2,
            "onz": np.ones((128, 8), bf),
            "one64": np.ones((1, 64), np.float32),
        })
    return in_maps


def kernel(x, Q_w, Q_b, K_w, K_b, V_w, V_b, O_w, O_b, _trace=False):
    x = np.asarray(x, np.float32)
    args = [np.asarray(a, np.float32)
            for a in (Q_w, Q_b, K_w, K_b, V_w, V_b, O_w)]
    O_b = np.asarray(O_b, np.float32)

    nc = _build()
    from concourse.bass_utils import run_bass_kernel_spmd

    in_maps = _host_inputs(x, *args)
    res = run_bass_kernel_spmd(nc, in_maps, core_ids=list(range(8)),
                               trace=_trace)
    _cache["last_result"] = res
    out = np.empty((B, S, DM), np.float32)
    for b in range(B):
        out[b] = res.results[2 * b]["y"] + res.results[2 * b + 1]["y"] + O_b
    return out


if __name__ == "__main__":
    # quick self-run with random inputs
    rng = np.random.default_rng(0)
    x = rng.standard_normal((B, S, DM), dtype=np.float32)
    shp = dict(Q_w=(H, DM, DH), Q_b=(H, DH), K_w=(H, DM, DH), K_b=(H, DH),
               V_w=(H, DM, DH), V_b=(H, DH), O_w=(H, DH, DM), O_b=(DM,))
    ins = {k: rng.standard_normal(v, dtype=np.float32) * 0.05
           for k, v in shp.items()}
    out = kernel(x, **ins)
    print("ran", out.shape, out.dtype)


# revision 27
# speedup vs baseline: 1.4932x; 1.0156x over previous
"""Multi-head causal attention (B=4, S=2048, H=16, Dh=64, Dm=1024) on 8
Trainium2 NeuronCores.

Sharding: core c handles batch b = c//2 and heads [8*(c%2), 8*(c%2)+8).
Each core computes its 8 heads' full attention + O-projection partial sum;
the host adds the two half-head partials per batch plus O_b.

v2 layout (all matmul inputs bf16, PSUM f32):
  - Inputs land in SBUF as bf16 (half the HBM traffic of v1); loads are
    spread over the sync + scalar HWDGE queues and emitted first.
  - One flat emission stream keeps the PE dense: the attention j-loop for
    pair 0 starts as soon as its q/k projections are in, and the remaining
    projection / O-projection matmul groups are drip-fed as fillers
    between j iterations so the PE never idles (holds the high p-state).
  - Causal narrowing: for diagonal k-tiles only columns >= o are computed
    (logits matmul, exp, S@V); the 128-wide staircase strip is masked with
    one DVE multiply (bf16, perf-mode eligible).
  - Softmax: exp (ACT) is the only Scalar-engine work.  Denominators come
    from the ones-column of the v tiles (M=65 S@V); normalization is
    DVE reciprocal on the PSUM denominator row -> gpsimd partition
    broadcast -> two DVE multiplies; head B is repacked to partitions
    64:127 with one SBUF->SBUF DMA so the O-projection contracts K=128.
  - O-projection writes PSUM straight to DRAM y via sync-queue DMA.
"""

import os
import sys

sys.path.insert(0, "/opt/trn_rl_repo")

import numpy as np

B, S, DM, H, DH = 4, 2048, 1024, 16, 64
HPC = 8          # heads per core
NPAIR = HPC // 2
PB = 512         # q block width
NQP = S // PB    # 4 q blocks
MT = DM // 128   # 8 m-tiles
NKT = S // 128   # 16 k tiles

_cache = {}


def _split_multi_waits(nc, mybir):
    # This container's walrus rejects >1 sync wait per instruction
    # ("Too many sync wait commands").  Move extra waits onto same-engine
    # NoOps right before the instruction; per-engine program order makes
    # this equivalent.
    ctr = 0
    for fn in nc.m.functions:
        for blk in fn.blocks:
            insts = list(blk.instructions)
            new_insts = []
            changed = False
            for inst in insts:
                si = getattr(inst, "sync_info", None)
                waits = list(si.on_wait) if (si is not None and si.on_wait) else []
                if len(waits) > 1:
                    changed = True
                    for w in waits[:-1]:
                        ctr += 1
                        new_insts.append(
                            mybir.InstNoOp(
                                name=f"waitsplit-{ctr}",
                                engine=inst.engine,
                                ins=[],
                                outs=[],
                                sync_info=mybir.SyncInfo(on_wait=[w], on_update=[]),
                            )
                        )
                    si.on_wait = [waits[-1]]
                new_insts.append(inst)
            if changed:
                blk.instructions = new_insts


def _patch_tile_drain(tile_mod, bass_mod):
    # Same walrus limitation hits the Tile kernel-tail drain (one wait per
    # ticked proc).  Chain the waits through single-wait sync NoOps.
    from concourse.vector_clock import ScopedClock, VectorClock

    def _drain_and_barrier(self, tick_clock, wait_clock):
        gc = tick_clock.global_clock
        n = len(gc)
        ticks = [gc[i] for i in range(n)]
        for p in [i for i in range(n) if ticks[i] > 0]:
            nop = self.nc.sync.nop(nofuse=True, hint="drain_wait_split")
            vc = VectorClock([ticks[j] if j == p else 0 for j in range(n)])
            wait_clock.add_sem_waits(nop.ins, ScopedClock({None: vc}))
        self.nc.sync.drain()
        self.nc.all_engine_barrier()
        assert self.sems is not None
        popped = self.nc._tile_sem_poison_stack.pop()
        assert popped is self._sem_poison
        self.nc.clear_and_free_semaphores(list(self.sems.allocated().values()))
        self.nc.all_engine_barrier()

    tile_mod.TileContext._drain_and_barrier = _drain_and_barrier


def _build():
    if "nc" in _cache:
        return _cache["nc"]

    import concourse.bass as bass
    import concourse.mybir as mybir
    import concourse.tile as tile

    _patch_tile_drain(tile, bass)

    f32 = mybir.dt.float32
    f32r = mybir.dt.float32r
    bf16 = mybir.dt.bfloat16
    Exp = mybir.ActivationFunctionType.Exp
    Ln = mybir.ActivationFunctionType.Ln

    nc = bass.Bass()
    xT = nc.dram_tensor("xT", [DM, S], bf16, kind="ExternalInput")
    Wq = nc.dram_tensor("Wq", [DM, 512], bf16, kind="ExternalInput")
    Wk = nc.dram_tensor("Wk", [DM, 512], bf16, kind="ExternalInput")
    Wv = nc.dram_tensor("Wv", [DM, 512], bf16, kind="ExternalInput")
    Wo = nc.dram_tensor("Wo", [512, DM], bf16, kind="ExternalInput")
    qkb = nc.dram_tensor("qkb", [128, 8], f32, kind="ExternalInput")
    vbb = nc.dram_tensor("vbb", [128, 512], f32, kind="ExternalInput")
    stair2 = nc.dram_tensor("stair2", [128, 256], bf16, kind="ExternalInput")
    onz = nc.dram_tensor("onz", [128, 8], bf16, kind="ExternalInput")
    one64 = nc.dram_tensor("one64", [1, 64], f32r, kind="ExternalInput")
    y = nc.dram_tensor("y", [S, DM], f32, kind="ExternalOutput")
    dd1s = [nc.dram_tensor(f"dd1_{k}", [1024], f32, kind="Internal")
            for k in range(2)]
    dd2s = [nc.dram_tensor(f"dd2_{k}", [1024], f32, kind="Internal")
            for k in range(2)]

    with tile.TileContext(nc) as tc:
        with nc.allow_low_precision(reason="bf16 tiles feeding the PE"), \
             tc.tile_pool(name="mp", bufs=1) as mp, \
             tc.tile_pool(name="sp", bufs=1) as sp, \
             tc.tile_pool(name="pp", bufs=1, space="PSUM") as pp:

            # ---- constants (gpsimd queue; tiny) ----
            qkb_sb = mp.tile([128, 8], f32, tag="qkb")
            nc.gpsimd.dma_start(qkb_sb[:], qkb[:])
            vbb_sb = mp.tile([128, 512], f32, tag="vbb")
            nc.gpsimd.dma_start(vbb_sb[:], vbb[:])
            stair_sb = mp.tile([128, 256], bf16, tag="stair")
            nc.gpsimd.dma_start(stair_sb[:], stair2[:])
            onz_sb = mp.tile([128, 8], bf16, tag="onz")
            nc.gpsimd.dma_start(onz_sb[:], onz[:])
            one64_sb = mp.tile([1, 64], f32r, tag="one64")
            nc.gpsimd.dma_start(one64_sb[:], one64[:])

            # ---- input loads.  x and wv first at full bandwidth (the
            # first V-proj group needs every xt m-tile); wq/wk/wo queue
            # BEHIND them on the same rings so they don't steal HBM ----
            wv = []
            for m in range(MT):
                w = mp.tile([128, 512], bf16, tag=f"wv{m}")
                nc.scalar.dma_start(w[:], Wv[m * 128:(m + 1) * 128, :])
                wv.append(w)
            xt = []
            for m in range(MT):
                t = mp.tile([128, S], bf16, tag=f"xt{m}")
                eng = nc.sync if m % 2 == 0 else nc.scalar
                eng.dma_start(t[:], xT[m * 128:(m + 1) * 128, :])
                xt.append(t)
            wq = []
            wk = []
            for m in range(MT):
                w = mp.tile([128, 512], bf16, tag=f"wq{m}")
                nc.sync.dma_start(w[:], Wq[m * 128:(m + 1) * 128, :])
                wq.append(w)
                w = mp.tile([128, 512], bf16, tag=f"wk{m}")
                nc.scalar.dma_start(w[:], Wk[m * 128:(m + 1) * 128, :])
                wk.append(w)
            wo = []
            for pri in range(NPAIR):
                t = mp.tile([128, DM], bf16, tag=f"wo{pri}")
                nc.gpsimd.dma_start(t[:], Wo[pri * 128:(pri + 1) * 128, :])
                wo.append(t)

            # ---- persistent result tiles ----
            # v: [p, h*65+d] per 128-row k-tile; col 65h+64 = ones so the
            # merged S@V matmul (M=65) also produces the softmax denominator
            v_sb = [mp.tile([128, 520], bf16, tag=f"v{p}", name=f"v{p}")
                    for p in range(NKT)]
            qkT = {(t, pri): mp.tile([128, S], bf16, tag=f"{t}T{pri}",
                                     name=f"{t}T{pri}")
                   for t in ("q", "k") for pri in range(NPAIR)}
            at_sb = {(pri, i): mp.tile([128, 512], bf16, tag=f"at{pri}_{i}",
                                       name=f"at{pri}_{i}")
                     for pri in range(NPAIR) for i in range(NQP)}

            # ---- filler units (each ~0.9-1.9us of PE work) ----
            def unit_vproj(p):
                def emit():
                    ps = pp.tile([128, 512], f32, tag="proj", bufs=2)
                    for m in range(MT):
                        nc.tensor.matmul(
                            ps[:], xt[m][:, p * 128:(p + 1) * 128], wv[m][:],
                            start=(m == 0), stop=(m == MT - 1))
                    vt = v_sb[p]
                    nc.vector.tensor_add(
                        vt.rearrange("p (h c) -> p h c", c=65)[:, :, 0:64],
                        ps.rearrange("p (h c) -> p h c", c=64),
                        vbb_sb.rearrange("p (h c) -> p h c", c=64))
                    nc.gpsimd.tensor_copy(
                        vt.rearrange("p (h c) -> p h c", c=65)[:, :, 64:65],
                        onz_sb.rearrange("p (h c) -> p h c", c=1))
                return emit

            def unit_qkproj(ti, pri, pb):
                def emit():
                    W = wq if ti == 0 else wk
                    out = qkT[("q" if ti == 0 else "k", pri)]
                    ps = pp.tile([128, 512], f32, tag="proj", bufs=2)
                    for m in range(MT):
                        nc.tensor.matmul(
                            ps[:], W[m][:, pri * 128:(pri + 1) * 128],
                            xt[m][:, pb * 512:(pb + 1) * 512],
                            start=(m == 0), stop=(m == MT - 1))
                    nc.vector.tensor_scalar_add(
                        out[:, pb * 512:(pb + 1) * 512], ps[:],
                        qkb_sb[:, 4 * ti + pri:4 * ti + pri + 1])
                return emit

            def unit_oproj(i, pt, dm):
                def emit():
                    P = 4 * i + pt
                    ps = pp.tile([128, 512], f32, tag="proj", bufs=2)
                    for pri in range(NPAIR):
                        nc.tensor.matmul(
                            ps[:],
                            at_sb[(pri, i)][:, pt * 128:(pt + 1) * 128],
                            wo[pri][:, dm * 512:(dm + 1) * 512],
                            start=(pri == 0), stop=(pri == NPAIR - 1))
                    yt = sp.tile([128, 512], f32, tag="yt", bufs=6, name="yt")
                    nc.vector.tensor_copy(yt[:], ps[:])
                    nc.sync.dma_start(
                        y[P * 128:(P + 1) * 128, dm * 512:(dm + 1) * 512],
                        yt[:])
                return emit

            from collections import deque
            fillers = deque()   # (key, emit_fn, req); keys track emission
            chainq = deque()    # deferred normalization-chain ops
            emitted = set()
            chain_emitted = [0]

            def pop_chain():
                chainq.popleft()()
                chain_emitted[0] += 1

            def pop_filler():
                key, fn, req = fillers[0]
                # a filler may read tiles written by deferred chain ops;
                # force-emit the chain up to its snapshot first
                while chain_emitted[0] < req and chainq:
                    pop_chain()
                fillers.popleft()
                fn()
                emitted.add(key)

            def drain_until(keys):
                # engines run their queues in emission order, so a unit
                # producing data for block (pri, i) must be EMITTED before
                # the block's first consumer instruction
                while fillers and not keys <= emitted:
                    pop_filler()

            # preamble compute: v tiles 0..3 + pair-0 q/k block 0
            for p in range(4):
                unit_vproj(p)()
                emitted.add(("v", p))
            unit_qkproj(0, 0, 0)()
            unit_qkproj(1, 0, 0)()
            emitted.update({("q", 0, 0), ("k", 0, 0)})
            # everything else pair 0 needs, in dependency order for i=1..3
            fillers.extend((("v", p), unit_vproj(p), 0) for p in (4, 5))
            fillers.append((("q", 0, 1), unit_qkproj(0, 0, 1), 0))
            fillers.append((("k", 0, 1), unit_qkproj(1, 0, 1), 0))
            fillers.extend((("v", p), unit_vproj(p), 0)
                           for p in (6, 7, 8, 9))
            fillers.append((("q", 0, 2), unit_qkproj(0, 0, 2), 0))
            fillers.append((("k", 0, 2), unit_qkproj(1, 0, 2), 0))
            fillers.extend((("v", p), unit_vproj(p), 0)
                           for p in (10, 11, 12, 13))
            fillers.append((("q", 0, 3), unit_qkproj(0, 0, 3), 0))
            fillers.append((("k", 0, 3), unit_qkproj(1, 0, 3), 0))
            fillers.extend((("v", p), unit_vproj(p), 0) for p in (14, 15))

            # ---- attention: pair-outer, q-block inner ----
            for pri in range(NPAIR):
                if pri < NPAIR - 1:
                    for pb in range(NQP):
                        fillers.append((("q", pri + 1, pb),
                                        unit_qkproj(0, pri + 1, pb), 0))
                        fillers.append((("k", pri + 1, pb),
                                        unit_qkproj(1, pri + 1, pb), 0))
                qT = qkT[("q", pri)]
                kT = qkT[("k", pri)]
                for i in range(NQP):
                    kmax = 4 * (i + 1)
                    drain_until({("q", pri, i)}
                                | {("k", pri, pb) for pb in range(i + 1)}
                                | {("v", p) for p in range(kmax)})
                    if i == 0 and fillers:
                        pop_filler()
                    ad = pp.tile([65, 1024], f32, tag="ad", bufs=1)

                    def emit_logits(j):
                        o = (j - 4 * i) * 128 if j >= 4 * i else 0
                        ev = pp.tile([128, 1024], f32, tag="ev", bufs=2,
                                     name="ev")
                        for h in range(2):
                            nc.tensor.matmul(
                                ev[:, h * 512 + o:(h + 1) * 512],
                                kT[64 * h:64 * h + 64, j * 128:(j + 1) * 128],
                                qT[64 * h:64 * h + 64,
                                   i * 512 + o:(i + 1) * 512],
                                start=True, stop=True)
                        return ev, o

                    # logits run two j's ahead so the ACT exp stream never
                    # starves across interleaved filler matmuls
                    evq = {0: emit_logits(0)}
                    if kmax > 1:
                        evq[1] = emit_logits(1)
                    for j in range(kmax):
                        ev, o = evq.pop(j)
                        sc = sp.tile([128, 1024], bf16, tag="sc", bufs=8)
                        if o:
                            nc.scalar.activation(
                                sc.rearrange("p (h c) -> p h c",
                                             c=512)[:, :, o:],
                                ev.rearrange("p (h c) -> p h c",
                                             c=512)[:, :, o:],
                                Exp, scale=0.125)
                        else:
                            nc.scalar.activation(sc[:], ev[:], Exp,
                                                 scale=0.125)
                        if j >= 4 * i:
                            # staircase mask on the 128-wide diagonal strip
                            # (both heads in one DVE multiply)
                            strip = sc.rearrange(
                                "p (h c) -> p h c", c=512)[:, :, o:o + 128]
                            nc.vector.tensor_mul(
                                strip, strip,
                                stair_sb.rearrange("p (h c) -> p h c", c=128))
                        st = (j == 0)
                        sp_ = (j == kmax - 1)
                        vt = v_sb[j]
                        for h in range(2):
                            lh = 2 * pri + h
                            nc.tensor.matmul(
                                ad[0:65, h * 512 + o:(h + 1) * 512],
                                vt[:, lh * 65:lh * 65 + 65],
                                sc[:, h * 512 + o:(h + 1) * 512],
                                start=st, stop=sp_, skip_group_check=True)
                        if j + 2 < kmax:
                            evq[j + 2] = emit_logits(j + 2)
                        if chainq:
                            pop_chain()
                        if fillers and (j % 2 == 1):
                            pop_filler()
                    # normalization: evacuate ad via ACT (releases the PSUM
                    # bank fast), then reciprocal + broadcast of the
                    # denominator row, then two multiplies + head-B repack
                    final = (pri == NPAIR - 1 and i == NQP - 1)
                    adc = sp.tile([65, 1024], f32, tag="adc", bufs=4,
                                  name="adc")
                    nc.scalar.copy(adc[:], ad[:, :])
                    at = at_sb[(pri, i)]
                    tmp = sp.tile([64, 512], bf16, tag="tmp", bufs=4)
                    # ACT hardware reciprocal of the denominator row (in the
                    # exp stream, right after the adc evacuation), then a PE
                    # outer-product broadcast and two DVE multiplies.  The
                    # PE/DVE ops are deferred into the next block's j-loop
                    # so they never head-of-line-block their engine queues.
                    # 1/den as exp(-ln(den)): Ln and Exp live in the same
                    # ACT table set as the softmax exp, so no table reloads
                    lnt = sp.tile([1, 1024], f32, tag="lnt", bufs=2,
                                  name="lnt")
                    nc.scalar.activation(lnt[:], adc[64:65, :], Ln)
                    rec = sp.tile([1, 1024], f32r, tag="rec", bufs=2,
                                  name="rec")
                    nc.scalar.activation(rec[:], lnt[:], Exp, scale=-1.0)

                    def chain_ops(adc=adc, at=at, tmp=tmp, rec=rec):
                        bch = [pp.tile([128, 512], f32, tag="proj", bufs=2,
                                       name=f"bc{hh}") for hh in range(2)]

                        def bc_mm(hh):
                            return lambda: nc.tensor.matmul(
                                bch[hh][0:64, :], one64_sb[:],
                                rec[:, hh * 512:(hh + 1) * 512],
                                start=True, stop=True)
                        yield bc_mm(0)
                        yield bc_mm(1)
                        yield lambda: nc.vector.tensor_mul(
                            at[0:64, :], adc[0:64, 0:512], bch[0][0:64, :])
                        yield lambda: nc.vector.tensor_mul(
                            tmp[:], adc[0:64, 512:1024], bch[1][0:64, :])
                        yield lambda: nc.sync.dma_start(at[64:128, :],
                                                        tmp[:])

                    chainq.extend(chain_ops())
                    if pri == NPAIR - 1:
                        req = chain_emitted[0] + len(chainq)
                        for dm in range(2):
                            for pt in range(4):
                                fillers.append((("o", i, pt, dm),
                                                unit_oproj(i, pt, dm), req))
            while chainq:
                pop_chain()
            while fillers:
                pop_filler()

    _split_multi_waits(nc, mybir)
    _cache["nc"] = nc
    return nc


def _host_inputs(x, Q_w, Q_b, K_w, K_b, V_w, V_b, O_w):
    import ml_dtypes
    bf = ml_dtypes.bfloat16
    stair = (np.arange(128)[:, None] <= np.arange(128)[None, :]).astype(bf)
    stair2 = np.concatenate([stair, stair], axis=1)
    in_maps = []
    for c in range(8):
        b, hs = c // 2, HPC * (c % 2)
        he = hs + HPC
        qb = Q_b[hs:he].reshape(512).astype(np.float32)
        kb = K_b[hs:he].reshape(512).astype(np.float32)
        qkb = np.zeros((128, 8), np.float32)
        for pri in range(NPAIR):
            qkb[:, pri] = qb[pri * 128:(pri + 1) * 128]
            qkb[:, 4 + pri] = kb[pri * 128:(pri + 1) * 128]
        in_maps.append({
            "xT": np.ascontiguousarray(x[b].T).astype(bf),
            "Wq": np.ascontiguousarray(
                Q_w[hs:he].transpose(1, 0, 2).reshape(DM, 512)).astype(bf),
            "Wk": np.ascontiguousarray(
                K_w[hs:he].transpose(1, 0, 2).reshape(DM, 512)).astype(bf),
            "Wv": np.ascontiguousarray(
                V_w[hs:he].transpose(1, 0, 2).reshape(DM, 512)).astype(bf),
            "Wo": np.ascontiguousarray(O_w[hs:he].reshape(512, DM)).astype(bf),
            "qkb": qkb,
            "vbb": np.tile(V_b[hs:he].reshape(1, 512), (128, 1)).astype(
                np.float32),
            "stair2": stair2,
            "onz": np.ones((128, 8), bf),
            "one64": np.ones((1, 64), np.float32),
        })
    return in_maps


def kernel(x, Q_w, Q_b, K_w, K_b, V_w, V_b, O_w, O_b, _trace=False):
    x = np.asarray(x, np.float32)
    args = [np.asarray(a, np.float32)
            for a in (Q_w, Q_b, K_w, K_b, V_w, V_b, O_w)]
    O_b = np.asarray(O_b, np.float32)

    nc = _build()
    from concourse.bass_utils import run_bass_kernel_spmd

    in_maps = _host_inputs(x, *args)
    res = run_bass_kernel_spmd(nc, in_maps, core_ids=list(range(8)),
                               trace=_trace)
    _cache["last_result"] = res
    out = np.empty((B, S, DM), np.float32)
    for b in range(B):
        out[b] = res.results[2 * b]["y"] + res.results[2 * b + 1]["y"] + O_b
    return out


if __name__ == "__main__":
    # quick self-run with random inputs
    rng = np.random.default_rng(0)
    x = rng.standard_normal((B, S, DM), dtype=np.float32)
    shp = dict(Q_w=(H, DM, DH), Q_b=(H, DH), K_w=(H, DM, DH), K_b=(H, DH),
               V_w=(H, DM, DH), V_b=(H, DH), O_w=(H, DH, DM), O_b=(DM,))
    ins = {k: rng.standard_normal(v, dtype=np.float32) * 0.05
           for k, v in shp.items()}
    out = kernel(x, **ins)
    print("ran", out.shape, out.dtype)


# revision 28
# speedup vs baseline: 1.5833x; 1.0603x over previous
"""Multi-head causal attention (B=4, S=2048, H=16, Dh=64, Dm=1024) on 8
Trainium2 NeuronCores.

Sharding: core c handles batch b = c//2 and heads [8*(c%2), 8*(c%2)+8).
Each core computes its 8 heads' full attention + O-projection partial sum;
the host adds the two half-head partials per batch plus O_b.

v2 layout (all matmul inputs bf16, PSUM f32):
  - Inputs land in SBUF as bf16 (half the HBM traffic of v1); loads are
    spread over the sync + scalar HWDGE queues and emitted first.
  - One flat emission stream keeps the PE dense: the attention j-loop for
    pair 0 starts as soon as its q/k projections are in, and the remaining
    projection / O-projection matmul groups are drip-fed as fillers
    between j iterations so the PE never idles (holds the high p-state).
  - Causal narrowing: for diagonal k-tiles only columns >= o are computed
    (logits matmul, exp, S@V); the 128-wide staircase strip is masked with
    one DVE multiply (bf16, perf-mode eligible).
  - Softmax: exp (ACT) is the only Scalar-engine work.  Denominators come
    from the ones-column of the v tiles (M=65 S@V); normalization is
    DVE reciprocal on the PSUM denominator row -> gpsimd partition
    broadcast -> two DVE multiplies; head B is repacked to partitions
    64:127 with one SBUF->SBUF DMA so the O-projection contracts K=128.
  - O-projection writes PSUM straight to DRAM y via sync-queue DMA.
"""

import os
import sys

sys.path.insert(0, "/opt/trn_rl_repo")

import numpy as np

B, S, DM, H, DH = 4, 2048, 1024, 16, 64
HPC = 8          # heads per core
NPAIR = HPC // 2
PB = 512         # q block width
NQP = S // PB    # 4 q blocks
MT = DM // 128   # 8 m-tiles
NKT = S // 128   # 16 k tiles

_cache = {}


def _split_multi_waits(nc, mybir):
    # This container's walrus rejects >1 sync wait per instruction
    # ("Too many sync wait commands").  Move extra waits onto same-engine
    # NoOps right before the instruction; per-engine program order makes
    # this equivalent.
    ctr = 0
    for fn in nc.m.functions:
        for blk in fn.blocks:
            insts = list(blk.instructions)
            new_insts = []
            changed = False
            for inst in insts:
                si = getattr(inst, "sync_info", None)
                waits = list(si.on_wait) if (si is not None and si.on_wait) else []
                if len(waits) > 1:
                    changed = True
                    for w in waits[:-1]:
                        ctr += 1
                        new_insts.append(
                            mybir.InstNoOp(
                                name=f"waitsplit-{ctr}",
                                engine=inst.engine,
                                ins=[],
                                outs=[],
                                sync_info=mybir.SyncInfo(on_wait=[w], on_update=[]),
                            )
                        )
                    si.on_wait = [waits[-1]]
                new_insts.append(inst)
            if changed:
                blk.instructions = new_insts


def _patch_tile_drain(tile_mod, bass_mod):
    # Same walrus limitation hits the Tile kernel-tail drain (one wait per
    # ticked proc).  Chain the waits through single-wait sync NoOps.
    from concourse.vector_clock import ScopedClock, VectorClock

    def _drain_and_barrier(self, tick_clock, wait_clock):
        gc = tick_clock.global_clock
        n = len(gc)
        ticks = [gc[i] for i in range(n)]
        for p in [i for i in range(n) if ticks[i] > 0]:
            nop = self.nc.sync.nop(nofuse=True, hint="drain_wait_split")
            vc = VectorClock([ticks[j] if j == p else 0 for j in range(n)])
            wait_clock.add_sem_waits(nop.ins, ScopedClock({None: vc}))
        self.nc.sync.drain()
        self.nc.all_engine_barrier()
        assert self.sems is not None
        popped = self.nc._tile_sem_poison_stack.pop()
        assert popped is self._sem_poison
        self.nc.clear_and_free_semaphores(list(self.sems.allocated().values()))
        self.nc.all_engine_barrier()

    tile_mod.TileContext._drain_and_barrier = _drain_and_barrier


def _build():
    if "nc" in _cache:
        return _cache["nc"]

    import concourse.bass as bass
    import concourse.mybir as mybir
    import concourse.tile as tile

    _patch_tile_drain(tile, bass)

    f32 = mybir.dt.float32
    f32r = mybir.dt.float32r
    bf16 = mybir.dt.bfloat16
    Exp = mybir.ActivationFunctionType.Exp
    Ln = mybir.ActivationFunctionType.Ln

    nc = bass.Bass()
    xT = nc.dram_tensor("xT", [DM, S], bf16, kind="ExternalInput")
    Wq = nc.dram_tensor("Wq", [DM, 512], bf16, kind="ExternalInput")
    Wk = nc.dram_tensor("Wk", [DM, 512], bf16, kind="ExternalInput")
    Wv = nc.dram_tensor("Wv", [DM, 512], bf16, kind="ExternalInput")
    Wo = nc.dram_tensor("Wo", [512, DM], bf16, kind="ExternalInput")
    qkb = nc.dram_tensor("qkb", [128, 8], f32, kind="ExternalInput")
    vbb = nc.dram_tensor("vbb", [128, 512], f32, kind="ExternalInput")
    stair2 = nc.dram_tensor("stair2", [128, 256], bf16, kind="ExternalInput")
    onz = nc.dram_tensor("onz", [128, 8], bf16, kind="ExternalInput")
    one64 = nc.dram_tensor("one64", [1, 64], f32r, kind="ExternalInput")
    y = nc.dram_tensor("y", [S, DM], f32, kind="ExternalOutput")
    dd1s = [nc.dram_tensor(f"dd1_{k}", [1024], f32, kind="Internal")
            for k in range(2)]
    dd2s = [nc.dram_tensor(f"dd2_{k}", [1024], f32, kind="Internal")
            for k in range(2)]

    with tile.TileContext(nc) as tc:
        with nc.allow_low_precision(reason="bf16 tiles feeding the PE"), \
             tc.tile_pool(name="mp", bufs=1) as mp, \
             tc.tile_pool(name="sp", bufs=1) as sp, \
             tc.tile_pool(name="pp", bufs=1, space="PSUM") as pp:

            # ---- constants (gpsimd queue; tiny) ----
            qkb_sb = mp.tile([128, 8], f32, tag="qkb")
            nc.gpsimd.dma_start(qkb_sb[:], qkb[:])
            vbb_sb = mp.tile([128, 512], f32, tag="vbb")
            nc.gpsimd.dma_start(vbb_sb[:], vbb[:])
            stair_sb = mp.tile([128, 256], bf16, tag="stair")
            nc.gpsimd.dma_start(stair_sb[:], stair2[:])
            onz_sb = mp.tile([128, 8], bf16, tag="onz")
            nc.gpsimd.dma_start(onz_sb[:], onz[:])
            one64_sb = mp.tile([1, 64], f32r, tag="one64")
            nc.gpsimd.dma_start(one64_sb[:], one64[:])

            # ---- input loads.  x and wv first at full bandwidth (the
            # first V-proj group needs every xt m-tile); wq/wk/wo queue
            # BEHIND them on the same rings so they don't steal HBM ----
            wv = []
            for m in range(MT):
                w = mp.tile([128, 512], bf16, tag=f"wv{m}")
                nc.scalar.dma_start(w[:], Wv[m * 128:(m + 1) * 128, :])
                wv.append(w)
            xt = []
            for m in range(MT):
                t = mp.tile([128, S], bf16, tag=f"xt{m}")
                eng = nc.sync if m % 2 == 0 else nc.scalar
                eng.dma_start(t[:], xT[m * 128:(m + 1) * 128, :])
                xt.append(t)
            wq = []
            wk = []
            for m in range(MT):
                w = mp.tile([128, 512], bf16, tag=f"wq{m}")
                nc.sync.dma_start(w[:], Wq[m * 128:(m + 1) * 128, :])
                wq.append(w)
                w = mp.tile([128, 512], bf16, tag=f"wk{m}")
                nc.scalar.dma_start(w[:], Wk[m * 128:(m + 1) * 128, :])
                wk.append(w)
            wo = []
            for pri in range(NPAIR):
                t = mp.tile([128, DM], bf16, tag=f"wo{pri}")
                nc.gpsimd.dma_start(t[:], Wo[pri * 128:(pri + 1) * 128, :])
                wo.append(t)

            # ---- persistent result tiles ----
            # v: [p, h*65+d] per 128-row k-tile; col 65h+64 = ones so the
            # merged S@V matmul (M=65) also produces the softmax denominator
            v_sb = [mp.tile([128, 520], bf16, tag=f"v{p}", name=f"v{p}")
                    for p in range(NKT)]
            qkT = {(t, pri): mp.tile([128, S], bf16, tag=f"{t}T{pri}",
                                     name=f"{t}T{pri}")
                   for t in ("q", "k") for pri in range(NPAIR)}
            at_sb = {(pri, i): mp.tile([128, 512], bf16, tag=f"at{pri}_{i}",
                                       name=f"at{pri}_{i}")
                     for pri in range(NPAIR) for i in range(NQP)}

            # ---- filler units (each ~0.9-1.9us of PE work) ----
            def unit_vproj(p):
                def emit():
                    ps = pp.tile([128, 512], f32, tag="proj", bufs=2)
                    for m in range(MT):
                        nc.tensor.matmul(
                            ps[:], xt[m][:, p * 128:(p + 1) * 128], wv[m][:],
                            start=(m == 0), stop=(m == MT - 1))
                    vt = v_sb[p]
                    nc.vector.tensor_add(
                        vt.rearrange("p (h c) -> p h c", c=65)[:, :, 0:64],
                        ps.rearrange("p (h c) -> p h c", c=64),
                        vbb_sb.rearrange("p (h c) -> p h c", c=64))
                    nc.gpsimd.tensor_copy(
                        vt.rearrange("p (h c) -> p h c", c=65)[:, :, 64:65],
                        onz_sb.rearrange("p (h c) -> p h c", c=1))
                return emit

            def unit_qkproj(ti, pri, pb):
                def emit():
                    W = wq if ti == 0 else wk
                    out = qkT[("q" if ti == 0 else "k", pri)]
                    ps = pp.tile([128, 512], f32, tag="proj", bufs=2)
                    for m in range(MT):
                        nc.tensor.matmul(
                            ps[:], W[m][:, pri * 128:(pri + 1) * 128],
                            xt[m][:, pb * 512:(pb + 1) * 512],
                            start=(m == 0), stop=(m == MT - 1))
                    nc.vector.tensor_scalar_add(
                        out[:, pb * 512:(pb + 1) * 512], ps[:],
                        qkb_sb[:, 4 * ti + pri:4 * ti + pri + 1])
                return emit

            def unit_oproj(i, pt, dm):
                def emit():
                    P = 4 * i + pt
                    ps = pp.tile([128, 512], f32, tag="proj", bufs=2)
                    for pri in range(NPAIR):
                        nc.tensor.matmul(
                            ps[:],
                            at_sb[(pri, i)][:, pt * 128:(pt + 1) * 128],
                            wo[pri][:, dm * 512:(dm + 1) * 512],
                            start=(pri == 0), stop=(pri == NPAIR - 1))
                    yt = sp.tile([128, 512], f32, tag="yt", bufs=6, name="yt")
                    nc.vector.tensor_copy(yt[:], ps[:])
                    nc.sync.dma_start(
                        y[P * 128:(P + 1) * 128, dm * 512:(dm + 1) * 512],
                        yt[:])
                return emit

            from collections import deque
            fillers = deque()   # (key, emit_fn, req); keys track emission
            chainq = deque()    # deferred normalization-chain ops
            emitted = set()
            chain_emitted = [0]

            def pop_chain():
                chainq.popleft()()
                chain_emitted[0] += 1

            def pop_filler():
                key, fn, req = fillers[0]
                # a filler may read tiles written by deferred chain ops;
                # force-emit the chain up to its snapshot first
                while chain_emitted[0] < req and chainq:
                    pop_chain()
                fillers.popleft()
                fn()
                emitted.add(key)

            def drain_until(keys):
                # engines run their queues in emission order, so a unit
                # producing data for block (pri, i) must be EMITTED before
                # the block's first consumer instruction
                while fillers and not keys <= emitted:
                    pop_filler()

            # preamble compute: v tiles 0..3 + pair-0 q/k block 0
            for p in range(4):
                unit_vproj(p)()
                emitted.add(("v", p))
            unit_qkproj(0, 0, 0)()
            unit_qkproj(1, 0, 0)()
            emitted.update({("q", 0, 0), ("k", 0, 0)})
            # everything else pair 0 needs, in dependency order for i=1..3
            fillers.extend((("v", p), unit_vproj(p), 0) for p in (4, 5))
            fillers.append((("q", 0, 1), unit_qkproj(0, 0, 1), 0))
            fillers.append((("k", 0, 1), unit_qkproj(1, 0, 1), 0))
            fillers.extend((("v", p), unit_vproj(p), 0)
                           for p in (6, 7, 8, 9))
            fillers.append((("q", 0, 2), unit_qkproj(0, 0, 2), 0))
            fillers.append((("k", 0, 2), unit_qkproj(1, 0, 2), 0))
            fillers.extend((("v", p), unit_vproj(p), 0)
                           for p in (10, 11, 12, 13))
            fillers.append((("q", 0, 3), unit_qkproj(0, 0, 3), 0))
            fillers.append((("k", 0, 3), unit_qkproj(1, 0, 3), 0))
            fillers.extend((("v", p), unit_vproj(p), 0) for p in (14, 15))

            # ---- attention: pair-outer, q-block inner ----
            for pri in range(NPAIR):
                if pri < NPAIR - 1:
                    for pb in range(NQP):
                        fillers.append((("q", pri + 1, pb),
                                        unit_qkproj(0, pri + 1, pb), 0))
                        fillers.append((("k", pri + 1, pb),
                                        unit_qkproj(1, pri + 1, pb), 0))
                qT = qkT[("q", pri)]
                kT = qkT[("k", pri)]
                for i in range(NQP):
                    kmax = 4 * (i + 1)
                    drain_until({("q", pri, i)}
                                | {("k", pri, pb) for pb in range(i + 1)}
                                | {("v", p) for p in range(kmax)})
                    if i == 0 and fillers:
                        pop_filler()
                    ad = pp.tile([65, 1024], f32, tag="ad", bufs=1)

                    def emit_logits(j):
                        o = (j - 4 * i) * 128 if j >= 4 * i else 0
                        ev = pp.tile([128, 1024], f32, tag="ev", bufs=2,
                                     name="ev")
                        for h in range(2):
                            nc.tensor.matmul(
                                ev[:, h * 512 + o:(h + 1) * 512],
                                kT[64 * h:64 * h + 64, j * 128:(j + 1) * 128],
                                qT[64 * h:64 * h + 64,
                                   i * 512 + o:(i + 1) * 512],
                                start=True, stop=True)
                        return ev, o

                    # logits run two j's ahead so the ACT exp stream never
                    # starves across interleaved filler matmuls
                    evq = {0: emit_logits(0)}
                    if kmax > 1:
                        evq[1] = emit_logits(1)
                    for j in range(kmax):
                        ev, o = evq.pop(j)
                        sc = sp.tile([128, 1024], bf16, tag="sc", bufs=8)
                        if o:
                            nc.scalar.activation(
                                sc.rearrange("p (h c) -> p h c",
                                             c=512)[:, :, o:],
                                ev.rearrange("p (h c) -> p h c",
                                             c=512)[:, :, o:],
                                Exp, scale=0.125)
                        else:
                            nc.scalar.activation(sc[:], ev[:], Exp,
                                                 scale=0.125)
                        if j >= 4 * i:
                            # staircase mask on the 128-wide diagonal strip
                            # (both heads in one DVE multiply)
                            strip = sc.rearrange(
                                "p (h c) -> p h c", c=512)[:, :, o:o + 128]
                            nc.vector.tensor_mul(
                                strip, strip,
                                stair_sb.rearrange("p (h c) -> p h c", c=128))
                        st = (j == 0)
                        sp_ = (j == kmax - 1)
                        vt = v_sb[j]
                        for h in range(2):
                            lh = 2 * pri + h
                            nc.tensor.matmul(
                                ad[0:65, h * 512 + o:(h + 1) * 512],
                                vt[:, lh * 65:lh * 65 + 65],
                                sc[:, h * 512 + o:(h + 1) * 512],
                                start=st, stop=sp_, skip_group_check=True)
                        if j + 2 < kmax:
                            evq[j + 2] = emit_logits(j + 2)
                        if chainq:
                            pop_chain()
                        if fillers and (j % 2 == 1):
                            pop_filler()
                    # normalization: evacuate ad via DVE (releases the PSUM
                    # bank fast and keeps the ACT exp stream uninterrupted
                    # across the block boundary); every other chain op is
                    # deferred into the next block's j-loop so it never
                    # head-of-line-blocks its engine queue.
                    adc = sp.tile([65, 1024], f32, tag="adc", bufs=4,
                                  name="adc")
                    nc.vector.tensor_copy(adc[:], ad[:, :])
                    at = at_sb[(pri, i)]
                    tmp = sp.tile([64, 512], bf16, tag="tmp", bufs=4)
                    # 1/den as exp(-ln(den)): Ln and Exp live in the same
                    # ACT table set as the softmax exp, so no table reloads
                    lnt = sp.tile([1, 1024], f32, tag="lnt", bufs=2,
                                  name="lnt")
                    rec = sp.tile([1, 1024], f32r, tag="rec", bufs=2,
                                  name="rec")

                    def chain_ops(adc=adc, at=at, tmp=tmp, rec=rec, lnt=lnt):
                        bch = [pp.tile([128, 512], f32, tag="proj", bufs=2,
                                       name=f"bc{hh}") for hh in range(2)]

                        def bc_mm(hh):
                            return lambda: nc.tensor.matmul(
                                bch[hh][0:64, :], one64_sb[:],
                                rec[:, hh * 512:(hh + 1) * 512],
                                start=True, stop=True)
                        yield lambda: nc.scalar.activation(
                            lnt[:], adc[64:65, :], Ln)
                        yield lambda: nc.scalar.activation(
                            rec[:], lnt[:], Exp, scale=-1.0)
                        yield bc_mm(0)
                        yield bc_mm(1)
                        yield lambda: nc.vector.tensor_mul(
                            at[0:64, :], adc[0:64, 0:512], bch[0][0:64, :])
                        yield lambda: nc.vector.tensor_mul(
                            tmp[:], adc[0:64, 512:1024], bch[1][0:64, :])
                        yield lambda: nc.sync.dma_start(at[64:128, :],
                                                        tmp[:])

                    chainq.extend(chain_ops())
                    if pri == NPAIR - 1:
                        req = chain_emitted[0] + len(chainq)
                        for dm in range(2):
                            for pt in range(4):
                                fillers.append((("o", i, pt, dm),
                                                unit_oproj(i, pt, dm), req))
            while chainq:
                pop_chain()
            while fillers:
                pop_filler()

    _split_multi_waits(nc, mybir)
    _cache["nc"] = nc
    return nc


def _host_inputs(x, Q_w, Q_b, K_w, K_b, V_w, V_b, O_w):
    import ml_dtypes
    bf = ml_dtypes.bfloat16
    stair = (np.arange(128)[:, None] <= np.arange(128)[None, :]).astype(bf)
    stair2 = np.concatenate([stair, stair], axis=1)
    in_maps = []
    for c in range(8):
        b, hs = c // 2, HPC * (c % 2)
        he = hs + HPC
        qb = Q_b[hs:he].reshape(512).astype(np.float32)
        kb = K_b[hs:he].reshape(512).astype(np.float32)
        qkb = np.zeros((128, 8), np.float32)
        for pri in range(NPAIR):
            qkb[:, pri] = qb[pri * 128:(pri + 1) * 128]
            qkb[:, 4 + pri] = kb[pri * 128:(pri + 1) * 128]
        in_maps.append({
            "xT": np.ascontiguousarray(x[b].T).astype(bf),
            "Wq": np.ascontiguousarray(
                Q_w[hs:he].transpose(1, 0, 2).reshape(DM, 512)).astype(bf),
            "Wk": np.ascontiguousarray(
                K_w[hs:he].transpose(1, 0, 2).reshape(DM, 512)).astype(bf),
            "Wv": np.ascontiguousarray(
                V_w[hs:he].transpose(1, 0, 2).reshape(DM, 512)).astype(bf),
            "Wo": np.ascontiguousarray(O_w[hs:he].reshape(512, DM)).astype(bf),
            "qkb": qkb,
            "vbb": np.tile(V_b[hs:he].reshape(1, 512), (128, 1)).astype(
                np.float32),
            "stair2": stair2,
            "onz": np.ones((128, 8), bf),
            "one64": np.ones((1, 64), np.float32),
        })
    return in_maps


def kernel(x, Q_w, Q_b, K_w, K_b, V_w, V_b, O_w, O_b, _trace=False):
    x = np.asarray(x, np.float32)
    args = [np.asarray(a, np.float32)
            for a in (Q_w, Q_b, K_w, K_b, V_w, V_b, O_w)]
    O_b = np.asarray(O_b, np.float32)

    nc = _build()
    from concourse.bass_utils import run_bass_kernel_spmd

    in_maps = _host_inputs(x, *args)
    res = run_bass_kernel_spmd(nc, in_maps, core_ids=list(range(8)),
                               trace=_trace)
    _cache["last_result"] = res
    out = np.empty((B, S, DM), np.float32)
    for b in range(B):
        out[b] = res.results[2 * b]["y"] + res.results[2 * b + 1]["y"] + O_b
    return out


if __name__ == "__main__":
    # quick self-run with random inputs
    rng = np.random.default_rng(0)
    x = rng.standard_normal((B, S, DM), dtype=np.float32)
    shp = dict(Q_w=(H, DM, DH), Q_b=(H, DH), K_w=(H, DM, DH), K_b=(H, DH),
               V_w=(H, DM, DH), V_b=(H, DH), O_w=(H, DH, DM), O_b=(DM,))
    ins = {k: rng.standard_normal(v, dtype=np.float32) * 0.05
           for k, v in shp.items()}
    out = kernel(x, **ins)
    print("ran", out.shape, out.dtype)
